# revision 5
# baseline (speedup 1.0000x reference)
"""Distributed Trainium2 kernel for a dense transformer block (v2).

Sharding (8 cores, one chip):
  - Each core owns S=512 of the 4096 tokens (sequence-parallel LN/FFN/residual)
    and one head-pair (2 of 16 heads) for attention.
  - QKV projections computed locally on own tokens for ALL heads (fp8
    DoubleRow), then ONE fused AllToAll redistributes q/k/v to head-owners.
  - Attention per head-pair over all tokens; softmax denominator folded into
    the AV matmul via a ones-column on V; AV runs fp8 DoubleRow over key-tile
    pairs; the scores->exp->AV chain is software-pipelined so the PE never
    waits on the activation engine.  Normalized outputs return to
    token-owners via a second (small) fp8 AllToAll.
  - Wo runs fp8 DoubleRow; the FFN runs bf16 (fp8 there fails the accuracy
    budget) with weights streamed from DRAM in 512KB chunks on the gpsimd
    DMA queue.
"""

import sys

sys.path.insert(0, "/opt/trn_rl_repo")

import numpy as np

import concourse.bacc as bacc
import concourse.bass as bass
import concourse.tile as tile
from concourse import mybir
from concourse.masks import make_identity

F32 = mybir.dt.float32
BF16 = mybir.dt.bfloat16
FP8 = mybir.dt.float8e4
AF = mybir.ActivationFunctionType
DR = mybir.MatmulPerfMode.DoubleRow

N_CORES = 8
B, T, D, H = 2, 2048, 1024, 16
HD = D // H            # 64
NTOK = B * T           # 4096
S = NTOK // N_CORES    # 512 tokens per core
F = 4 * D              # 4096
EPS = 1e-5
SCALE = float(D) ** -0.5
MASK_VAL = -30000.0
P = 128

KT = D // P            # 8 feature tiles
TT = S // P            # 4 token tiles in the shard
NW = N_CORES           # 8 global 512-token windows
FT = F // P            # 32 ffn-hidden tiles
CP = D // 256          # 4 contraction chunk-pairs over D
FJ = F // 256          # 16 chunk-pairs over F
BLK = 3 * P            # 384 rows per a2a1 block (q,k,v)

_CACHE = {}


def _build(n_chain=1, stub_cc=False):
    nc = bacc.Bacc("TRN2", target_bir_lowering=False, debug=False,
                   num_devices=N_CORES)

    x = nc.dram_tensor("x", [S, D], F32, kind="ExternalInput")
    wqkv8 = nc.dram_tensor("wqkv8", [3 * 512, 2048], FP8, kind="ExternalInput")
    wo8 = nc.dram_tensor("wo8", [512, 2048], FP8, kind="ExternalInput")
    w1b = nc.dram_tensor("w1b", [F // 2, 2 * D], BF16, kind="ExternalInput")
    w2b = nc.dram_tensor("w2b", [F // 2, 2 * D], BF16, kind="ExternalInput")
    bo = nc.dram_tensor("bo", [D], F32, kind="ExternalInput")
    b1 = nc.dram_tensor("b1", [F], F32, kind="ExternalInput")
    b2 = nc.dram_tensor("b2", [D], F32, kind="ExternalInput")
    ln1_g = nc.dram_tensor("ln1_g", [D], F32, kind="ExternalInput")
    ln1_b = nc.dram_tensor("ln1_b", [D], F32, kind="ExternalInput")
    ln2_g = nc.dram_tensor("ln2_g", [D], F32, kind="ExternalInput")
    ln2_b = nc.dram_tensor("ln2_b", [D], F32, kind="ExternalInput")
    y = nc.dram_tensor("y", [S, D], F32, kind="ExternalOutput")
    global _W
    _W = dict(wqkv8=wqkv8, wo8=wo8, w1b=w1b, w2b=w2b, bo=bo, b1=b1, b2=b2,
              ln1_g=ln1_g, ln1_b=ln1_b, ln2_g=ln2_g, ln2_b=ln2_b)

    with tile.TileContext(nc) as tc:
      with tc.tile_pool(name="xch", bufs=1) as xchp, \
           tc.tile_pool(name="gw", bufs=1) as gwp:
        xsets = [[xchp.tile([P, D], F32, tag=f"x{s}_{i}", name=f"x{s}_{i}")
                  for i in range(TT)] for s in range(2)]
        wsb = [gwp.tile([P, 8192], FP8, tag=f"w{pj}", name=f"w{pj}")
               for pj in range(3)]
        for pj in range(3):
            nc.gpsimd.dma_start(
                out=wsb[pj][:].rearrange("p (c n) -> p c n", c=CP),
                in_=wqkv8[pj * 512:(pj + 1) * 512, :].rearrange(
                    "(c p) n -> p c n", p=P))
        wos = [gwp.tile([P, 2048], FP8, tag=f"wos{c}", name=f"wos{c}")
               for c in range(CP)]
        for c in range(CP):
            nc.gpsimd.dma_start(out=wos[c][:], in_=wo8[c * P:(c + 1) * P, :])
        for _ci in range(n_chain):
            _emit_body(nc, tc,
                       x if _ci == 0 else None,
                       y if _ci == n_chain - 1 else None,
                       xsets[_ci % 2], xsets[(_ci + 1) % 2],
                       wsb, wos, _ci, stub_cc)

    nc.compile()
    return nc


def _emit_body(nc, tc, x, y, x_sb, x_out, wsb, wos, ci, stub_cc=False):
    wqkv8, wo8, w1b, w2b = _W["wqkv8"], _W["wo8"], _W["w1b"], _W["w2b"]
    bo, b1, b2 = _W["bo"], _W["b1"], _W["b2"]
    ln1_g, ln1_b = _W["ln1_g"], _W["ln1_b"]
    ln2_g, ln2_b = _W["ln2_g"], _W["ln2_b"]

    with tc.tile_pool(name="dram", bufs=1, space="DRAM") as dram, \
         tc.tile_pool(name="const", bufs=1) as const, \
         tc.tile_pool(name="persist", bufs=1) as persist:

        a2a1_in = dram.tile([NW * 2 * P, S], FP8, tag="a1i", name="a2a1_in")
        a2a1_out = dram.tile([NW * 2 * P, S], FP8, tag="a1o",
                             name="a2a1_out")
        a2av_in = dram.tile([NW * P, S], FP8, tag="avi", name="a2av_in")
        a2av_out = dram.tile([NW * P, S], FP8, tag="avo", name="a2av_out")
        a2a2_in = dram.tile([NW * P, S], FP8, tag="a2i", name="a2a2_in")
        a2a2_out = dram.tile([NW * P, S], FP8, tag="a2o", name="a2a2_out")

        # ---- constants ----
        ident_b = const.tile([P, P], BF16, tag="identb", name="ident_b")
        make_identity(nc, ident_b)

        eps_t = const.tile([P, 1], F32, tag="eps", name="eps_t")
        nc.vector.memset(eps_t[:], EPS)

        # triangular mask for the diagonal 128x128 sub-block:
        # m[r, q] = 0 if r <= q else MASK_VAL
        mask_tri = const.tile([P, P], F32, tag="mtri", name="mask_tri")
        nc.gpsimd.memset(mask_tri[:], 0.0)
        nc.gpsimd.affine_select(
            out=mask_tri[:], in_=mask_tri[:],
            compare_op=mybir.AluOpType.is_ge,
            fill=MASK_VAL, base=0,
            pattern=[[1, P]], channel_multiplier=-1,
        )

        g1_s = const.tile([P, KT], F32, tag="g1", name="g1_s")
        b1l_s = const.tile([P, KT], F32, tag="b1l", name="b1l_s")
        g2_s = const.tile([P, KT], F32, tag="g2", name="g2_s")
        b2l_s = const.tile([P, KT], F32, tag="b2l", name="b2l_s")
        nc.scalar.dma_start(out=g1_s[:],
                            in_=ln1_g.ap().rearrange("(k p) -> p k", p=P))
        nc.scalar.dma_start(out=b1l_s[:],
                            in_=ln1_b.ap().rearrange("(k p) -> p k", p=P))
        nc.scalar.dma_start(out=g2_s[:],
                            in_=ln2_g.ap().rearrange("(k p) -> p k", p=P))
        nc.scalar.dma_start(out=b2l_s[:],
                            in_=ln2_b.ap().rearrange("(k p) -> p k", p=P))

        b1_s = const.tile([P, FT], F32, tag="b1s", name="b1_s")
        nc.scalar.dma_start(out=b1_s[:],
                            in_=b1.ap().rearrange("(k p) -> p k", p=P))

        bo_bc = const.tile([P, D], F32, tag="bo_bc", name="bo_bc")
        b2_bc = const.tile([P, D], F32, tag="b2_bc", name="b2_bc")
        nc.scalar.dma_start(out=bo_bc[:], in_=bo.ap().partition_broadcast(P))
        nc.scalar.dma_start(out=b2_bc[:], in_=b2.ap().partition_broadcast(P))

        # persistent: r1; x shard tiles passed in (SBUF-resident chain)
        if x is not None:
            for i in range(TT):
                nc.sync.dma_start(out=x_sb[i][:],
                                  in_=x[i * P:(i + 1) * P, :])
        r1 = [persist.tile([P, D], F32, tag=f"r1_{i}", name=f"r1_{i}")
              for i in range(TT)]

        def layernorm_tiles(src_tiles, pool, out_name):
            """LN over the free axis -> normalized bf16 [t,d] tiles
            (gamma/beta folded in later at transpose-evict)."""
            out = []
            with tc.tile_pool(name=f"ln_{out_name}", bufs=2) as lnp:
                for i, xt in enumerate(src_tiles):
                    st = lnp.tile([P, 2, 6], F32, tag="stats", name="st")
                    xr = xt[:].rearrange("p (s f) -> p s f", s=2)
                    for sg in range(2):
                        nc.vector.bn_stats(out=st[:, sg, :], in_=xr[:, sg, :])
                    mv = lnp.tile([P, 2], F32, tag="mv", name="mv")
                    nc.vector.bn_aggr(out=mv[:], in_=st[:])
                    rstd = lnp.tile([P, 1], F32, tag="rstd", name="rstd")
                    nc.scalar.activation(out=rstd[:], in_=mv[:, 1:2],
                                         func=AF.Sqrt, bias=eps_t[:],
                                         scale=1.0)
                    nc.vector.reciprocal(out=rstd[:], in_=rstd[:])
                    o = pool.tile([P, D], BF16, tag=f"{out_name}{i}",
                                  name=f"{out_name}{i}")
                    nc.vector.tensor_scalar(
                        out=o[:], in0=xt[:],
                        scalar1=mv[:, 0:1], scalar2=rstd[:],
                        op0=mybir.AluOpType.subtract,
                        op1=mybir.AluOpType.mult,
                    )
                    out.append(o)
            return out

        def pair_view(t, width):
            """[128, 2*width] tile -> [128, 2, width] DoubleRow view."""
            return t[:].rearrange("p (two n) -> p two n", two=2)

        # ================= QKV phase =================
        with tc.tile_pool(name="qkvsb", bufs=1) as qkvsb:
            # LN1 + transpose, folding gamma/beta, writing fp8 pair tiles
            h8 = [qkvsb.tile([P, 1024], FP8, tag=f"h8_{c}", name=f"h8_{c}")
                  for c in range(CP)]
            with tc.tile_pool(name="xlnp", bufs=1) as xlnp:
                xln = layernorm_tiles(x_sb, xlnp, "xln")
                with tc.tile_pool(name="tr1p", bufs=4, space="PSUM") as trpp:
                    for i in range(TT):
                        for kt in range(KT):
                            pt = trpp.tile([P, P], BF16, tag="tr", name="pt")
                            nc.tensor.transpose(
                                pt[:], xln[i][:, kt * P:(kt + 1) * P],
                                ident_b[:])
                            nc.vector.tensor_scalar(
                                out=h8[kt // 2][:, (kt % 2) * 512 + i * P:
                                                (kt % 2) * 512 + (i + 1) * P],
                                in0=pt[:],
                                scalar1=g1_s[:, kt:kt + 1],
                                scalar2=b1l_s[:, kt:kt + 1],
                                op0=mybir.AluOpType.mult,
                                op1=mybir.AluOpType.add,
                            )

            def wqkv_view(pj, c, ob):
                v = wsb[pj][:].rearrange("p (c two n) -> p c two n",
                                         c=CP, two=2)
                return v[:, c, :, ob * P:(ob + 1) * P]

            with tc.tile_pool(name="qkvst", bufs=3) as qst, \
                 tc.tile_pool(name="qkvp", bufs=2, space="PSUM") as qkvp, \
                 tc.tile_pool(name="vtp", bufs=2, space="PSUM") as vtp:
                pend_v = []

                def emit_vtr():
                    stv, ob = pend_v.pop(0)
                    for j4 in range(4):
                        pv = vtp.tile([P, P], BF16, tag="pv", name="pv")
                        nc.tensor.transpose(pv[:], stv[:, j4 * P:(j4 + 1) * P],
                                            ident_b[:])
                        v8 = qst.tile([P, P], FP8, tag="v8", name="v8")
                        nc.vector.tensor_copy(v8[:], pv[:])
                        nc.sync.dma_start(
                            out=a2av_in[ob * P + 32 * j4:
                                        ob * P + 32 * (j4 + 1),
                                        :].rearrange("r (s f) -> (r s) f",
                                                     s=4),
                            in_=v8[:])

                for ob in range(NW):
                    pss = [qkvp.tile([P, 512], F32, tag=f"pp{pj}",
                                     name=f"pp{pj}") for pj in range(3)]
                    for c in range(CP):
                        for pj in range(3):
                            nc.tensor.matmul(
                                pss[pj][:],
                                wqkv_view(pj, c, ob),
                                pair_view(h8[c], 512),
                                start=(c == 0), stop=(c == CP - 1),
                                perf_mode=DR)
                    st = qst.tile([P, 1024], FP8, tag="st", name="st")
                    nc.scalar.copy(st[:, 0:512], pss[0][:])
                    nc.scalar.copy(st[:, 512:1024], pss[1][:])
                    nc.sync.dma_start(
                        out=a2a1_in[ob * 2 * P:(ob + 1) * 2 * P,
                                    :].rearrange(
                            "(blk p) c -> p blk c", p=P),
                        in_=st[:].rearrange("p (blk c) -> p blk c", blk=2))
                    stv = qst.tile([P, 512], BF16, tag="stv", name="stv")
                    nc.vector.tensor_copy(stv[:], pss[2][:])
                    pend_v.append((stv, ob))
                    if len(pend_v) > 1:
                        emit_vtr()
                while pend_v:
                    emit_vtr()

        if stub_cc:
            nc.sync.dma_start(out=a2a1_out[:, :], in_=a2a1_in[:, :])
            nc.sync.dma_start(out=a2av_out[:, :], in_=a2av_in[:, :])
        else:
            nc.gpsimd.collective_compute(
                "AllToAll", mybir.AluOpType.bypass,
                replica_groups=[list(range(N_CORES))],
                ins=[a2a1_in.opt()], outs=[a2a1_out.opt()],
            )
            nc.gpsimd.collective_compute(
                "AllToAll", mybir.AluOpType.bypass,
                replica_groups=[list(range(N_CORES))],
                ins=[a2av_in.opt()], outs=[a2av_out.opt()],
            )

        # ============ attention super-phase ============
        with tc.tile_pool(name="attnsb", bufs=1) as attnsb:
            # x + bo (residual base), done while collective runs
            for i in range(TT):
                nc.vector.tensor_add(out=x_sb[i][:], in0=x_sb[i][:],
                                     in1=bo_bc[:])

            # qkv_w[w]: [128, 1024] = qT | kT for window w
            qkv_w = [attnsb.tile([P, 2 * 512], FP8, tag=f"qkv{w}",
                                 name=f"qkv{w}") for w in range(NW)]
            # v_aug per head per key-tile pair: [128 keys, 2, 96]
            # (64 v cols + ones col + zero pad; DR stationary width must be
            # a multiple of 32)
            vgA = [attnsb.tile([P, 192], FP8, tag=f"vgA{i}", name=f"vgA{i}")
                   for i in range(16)]
            vgB = [attnsb.tile([P, 192], FP8, tag=f"vgB{i}", name=f"vgB{i}")
                   for i in range(16)]
            for i in range(16):
                for vg in (vgA[i], vgB[i]):
                    nc.vector.memset(vg[:], 0.0)
                    nc.vector.memset(vg[:, HD:HD + 1], 1.0)
                    nc.vector.memset(vg[:, 96 + HD:96 + HD + 1], 1.0)

            # prefetch all window loads up front; v goes straight into
            # the v_aug pair tiles (token-major already)
            for w in range(NW):
                nc.sync.dma_start(
                    out=qkv_w[w][:].rearrange("p (blk c) -> p blk c",
                                              blk=2),
                    in_=a2a1_out[w * 2 * P:(w + 1) * 2 * P, :].rearrange(
                        "(blk p) c -> p blk c", p=P))
                for si in range(4):
                    vsrc = a2av_out[w * P + 32 * si:
                                    w * P + 32 * (si + 1),
                                    :].rearrange("r (s f) -> (r s) f", s=4)
                    pi = w * 2 + si // 2
                    col = (si % 2) * 96
                    nc.gpsimd.dma_start(out=vgA[pi][:, col:col + HD],
                                        in_=vsrc[:, 0:HD])
                    nc.gpsimd.dma_start(out=vgB[pi][:, col:col + HD],
                                        in_=vsrc[:, HD:P])

            o_sbs = [attnsb.tile([65, 512], BF16, tag=f"osb{i}",
                                 name=f"osb{i}") for i in range(16)]
            with tc.tile_pool(name="scp", bufs=3, space="PSUM") as scp, \
                 tc.tile_pool(name="ptp", bufs=6) as ptp, \
                 tc.tile_pool(name="lop", bufs=1, space="PSUM") as lop, \
                 tc.tile_pool(name="nrm2", bufs=3) as nrm2:

                def attn_window(b, tcl):
                    tch = b * 4 + tcl
                    n_s = 4 * (tcl + 1)
                    np2 = n_s // 2
                    qT = qkv_w[tch]
                    o_ps = {}
                    for hh in range(2):
                        o_ps[hh] = lop.tile([96, 512], F32, tag=f"ops{hh}",
                                            name=f"o_ps{hh}")
                    pend = []

                    def emit_sc(j, hh):
                        rows = slice(hh * HD, (hh + 1) * HD)
                        sc = scp.tile([P, 1024], F32, tag="sc", name="sc")
                        diag = []
                        for half in range(2):
                            si = 2 * j + half
                            kw = b * 4 + si // 4
                            kc = 512 + (si % 4) * P
                            nc.tensor.matmul(
                                sc[:, half * 512:(half + 1) * 512],
                                qkv_w[kw][rows, kc:kc + P],
                                qT[rows, 0:512],
                                start=True, stop=True,
                                tile_position=(hh * HD, 0))
                            if si // 4 == tcl:
                                diag.append((half, (si % 4) * P))
                        p = ptp.tile([P, 1024], FP8, tag="p", name="p")
                        if not diag:
                            nc.scalar.activation(
                                out=p[:], in_=sc[:],
                                func=AF.Exp, scale=SCALE)
                        else:
                            for half, c0 in diag:
                                base = half * 512
                                nc.vector.tensor_add(
                                    out=sc[:, base + c0:base + c0 + P],
                                    in0=sc[:, base + c0:base + c0 + P],
                                    in1=mask_tri[:])
                                if c0:
                                    nc.vector.memset(
                                        p[:, base:base + c0], 0.0)
                                nc.scalar.activation(
                                    out=p[:, base + c0:base + 512],
                                    in_=sc[:, base + c0:base + 512],
                                    func=AF.Exp, scale=SCALE)
                        pend.append((p, j, hh))

                    def emit_av():
                        p, j, hh = pend.pop(0)
                        vg = vgA if hh == 0 else vgB
                        nc.tensor.matmul(
                            o_ps[hh][:], pair_view(vg[b * 8 + j], 96),
                            pair_view(p, 512),
                            start=(j == 0), stop=(j == np2 - 1),
                            perf_mode=DR)

                    for j in range(np2):
                        emit_sc(j, 0)
                        emit_sc(j, 1)
                        while len(pend) > 2:
                            emit_av()
                    while pend:
                        emit_av()
                    for hh in range(2):
                        osb = o_sbs[tch * 2 + hh]
                        nc.vector.tensor_copy(osb[:], o_ps[hh][0:65, :])

                def normalize_entry(idx):
                    osb = o_sbs[idx]
                    lf = nrm2.tile([65, 512], F32, tag="lf", name="lf")
                    nc.vector.tensor_copy(lf[64:65, :], osb[64:65, :])
                    l0 = nrm2.tile([1, 512], F32, tag="l0", name="l0")
                    nc.sync.dma_start(out=l0[:], in_=lf[64:65, :])
                    lr = nrm2.tile([1, 512], F32, tag="lr", name="lr")
                    nc.vector.reciprocal_approx_fast(out=lr[:], in_=l0[:])
                    lbc = nrm2.tile([HD, 512], F32, tag="lbc", name="lbc")
                    nc.gpsimd.partition_broadcast(lbc[:], lr[0:1, :])
                    o_n = nrm2.tile([HD, 512], FP8, tag="on", name="o_n")
                    nc.vector.tensor_mul(out=o_n[:], in0=osb[0:HD, :],
                                         in1=lbc[:])
                    nc.sync.dma_start(
                        out=a2a2_in[idx * HD:(idx + 1) * HD, :],
                        in_=o_n[:])

                for b in range(B):
                    for tcl in range(4):
                        attn_window(b, tcl)
                        # batch-0 normalize ships while batch-1 attention
                        # still runs
                        if b == 1 and tcl == 0:
                            for idx in range(8):
                                normalize_entry(idx)
                for idx in range(8, 16):
                    normalize_entry(idx)

            if stub_cc:
                nc.sync.dma_start(out=a2a2_out[:, :], in_=a2a2_in[:, :])
            else:
                nc.gpsimd.collective_compute(
                    "AllToAll", mybir.AluOpType.bypass,
                    replica_groups=[list(range(N_CORES))],
                    ins=[a2a2_in.opt()], outs=[a2a2_out.opt()],
                )

            # ---- Wo projection (DoubleRow) + residual ----
            with tc.tile_pool(name="wosb", bufs=1) as wosb, \
                 tc.tile_pool(name="wop", bufs=1, space="PSUM") as wop:
                o8 = [wosb.tile([P, 1024], FP8, tag=f"o8_{c}", name=f"o8_{c}")
                      for c in range(CP)]
                for c in range(CP):
                    nc.sync.dma_start(
                        out=o8[c][:].rearrange("p (two c) -> p two c", two=2),
                        in_=a2a2_out[c * 256:(c + 1) * 256, :].rearrange(
                            "(two p) c -> p two c", p=P))
                pso = [wop.tile([P, 512], F32, tag=f"wo{i}", name=f"wo{i}")
                       for i in range(8)]
                for c in range(CP):
                    for tt in range(TT):
                        lhs = pair_view(o8[c], 512)[:, :, tt * P:(tt + 1) * P]
                        for dc in range(2):
                            nc.tensor.matmul(
                                pso[tt * 2 + dc][:], lhs,
                                pair_view(wos[c], 1024)[:, :,
                                                        dc * 512:
                                                        (dc + 1) * 512],
                                start=(c == 0), stop=(c == CP - 1),
                                perf_mode=DR)
                for tt in range(TT):
                    for dc in range(2):
                        sl = slice(dc * 512, (dc + 1) * 512)
                        nc.vector.tensor_add(out=r1[tt][:, sl],
                                             in0=pso[tt * 2 + dc][:],
                                             in1=x_sb[tt][:, sl])

            # ---- LN2 + transpose (bf16, kt-major) ----
            h2T = [attnsb.tile([P, 512], BF16, tag=f"h2T{k}",
                               name=f"h2T{k}") for k in range(KT)]
            with tc.tile_pool(name="h2p", bufs=1) as h2p:
                h2 = layernorm_tiles(r1, h2p, "h2")
                with tc.tile_pool(name="tr2p", bufs=4, space="PSUM") as tr2p:
                    for i in range(TT):
                        for kt in range(KT):
                            pt2 = tr2p.tile([P, P], BF16, tag="tr2",
                                            name="pt2")
                            nc.tensor.transpose(
                                pt2[:], h2[i][:, kt * P:(kt + 1) * P],
                                ident_b[:])
                            nc.vector.tensor_scalar(
                                out=h2T[kt][:, i * P:(i + 1) * P],
                                in0=pt2[:],
                                scalar1=g2_s[:, kt:kt + 1],
                                scalar2=b2l_s[:, kt:kt + 1],
                                op0=mybir.AluOpType.mult,
                                op1=mybir.AluOpType.add,
                            )

            # r1 + b2 (residual base for ffn out)
            for i in range(TT):
                nc.vector.tensor_add(out=r1[i][:], in0=r1[i][:],
                                     in1=b2_bc[:])

            # ---- FFN (bf16, streamed weights) ----
            ff1 = [attnsb.tile([P, 512], BF16, tag=f"ff1_{k}",
                               name=f"ff1_{k}") for k in range(FT)]
            with tc.tile_pool(name="w1st", bufs=4) as w1st, \
                 tc.tile_pool(name="ff1pp", bufs=4, space="PSUM") as ff1pp:
                for j in range(FJ):
                    w1t = w1st.tile([P, 2048], BF16, tag="w1t", name="w1t")
                    nc.gpsimd.dma_start(out=w1t[:],
                                        in_=w1b[j * P:(j + 1) * P, :])
                    for half in range(2):
                        ft = 2 * j + half
                        ps = ff1pp.tile([P, 512], F32, tag="ff1", name="ps")
                        for kt in range(KT):
                            nc.tensor.matmul(
                                ps[:],
                                w1t[:, half * 1024 + kt * P:
                                    half * 1024 + (kt + 1) * P],
                                h2T[kt][:],
                                start=(kt == 0), stop=(kt == KT - 1))
                        nc.scalar.activation(
                            out=ff1[ft][:],
                            in_=ps[:], func=AF.Relu, bias=b1_s[:, ft:ft + 1])

            with tc.tile_pool(name="w2st", bufs=4) as w2st, \
                 tc.tile_pool(name="outp", bufs=4) as outp, \
                 tc.tile_pool(name="ff2p", bufs=1, space="PSUM") as ff2p:
                ps2 = [ff2p.tile([P, 512], F32, tag=f"ff2_{i}",
                                 name=f"ff2_{i}") for i in range(8)]
                w2ts = []
                for j in range(2):
                    w2t = w2st.tile([P, 2048], BF16, tag="w2t", name="w2t")
                    nc.scalar.dma_start(out=w2t[:],
                                        in_=w2b[j * P:(j + 1) * P, :])
                    w2ts.append(w2t)
                for j in range(FJ):
                    if j < 2:
                        w2t = w2ts[j]
                    else:
                        w2t = w2st.tile([P, 2048], BF16, tag="w2t",
                                        name="w2t")
                        nc.gpsimd.dma_start(out=w2t[:],
                                            in_=w2b[j * P:(j + 1) * P, :])
                    for half in range(2):
                        kt = 2 * j + half
                        for tt in range(TT):
                            lhs = ff1[kt][:, tt * P:(tt + 1) * P]
                            for dc in range(2):
                                nc.tensor.matmul(
                                    ps2[tt * 2 + dc][:], lhs,
                                    w2t[:, half * 1024 + dc * 512:
                                        half * 1024 + (dc + 1) * 512],
                                    start=(kt == 0), stop=(kt == FT - 1))
                for tt in range(TT):
                    for dc in range(2):
                        sl = slice(dc * 512, (dc + 1) * 512)
                        nc.vector.tensor_add(out=x_out[tt][:, sl],
                                             in0=ps2[tt * 2 + dc][:],
                                             in1=r1[tt][:, sl])
                    if y is not None:
                        nc.sync.dma_start(out=y[tt * P:(tt + 1) * P, :],
                                          in_=x_out[tt][:])


def _dr_pack(w, dt=None):
    """[K, N] -> DoubleRow pair layout [K/2, 2N]: row c*128+p,
    col i*N+n  holds  w[c*256 + i*128 + p, n]."""
    import ml_dtypes
    if dt is None:
        dt = ml_dtypes.float8_e4m3fn
    K, N = w.shape
    nch = K // 256
    t = w.reshape(nch, 2, 128, N).transpose(0, 2, 1, 3).reshape(K // 2, 2 * N)
    return np.ascontiguousarray(t.astype(dt))


def _w1_pack(w1):
    """[D, F] -> streamed lhsT pair tiles: row j*128+p,
    col half*1024 + kt*128 + c  holds  W1[kt*128+p, (2j+half)*128+c]."""
    import ml_dtypes
    t = w1.reshape(KT, P, FJ, 2, P).transpose(2, 1, 3, 0, 4).reshape(
        F // 2, 2 * D)
    return np.ascontiguousarray(t.astype(ml_dtypes.bfloat16))


def _shard_inputs(inputs):
    import ml_dtypes
    x = np.ascontiguousarray(
        np.asarray(inputs["x"], np.float32).reshape(NTOK, D))
    Wq = np.asarray(inputs["Wq"], np.float32).transpose(1, 0, 2).reshape(D, D)
    Wk = np.asarray(inputs["Wk"], np.float32).transpose(1, 0, 2).reshape(D, D)
    Wv = np.asarray(inputs["Wv"], np.float32).transpose(1, 0, 2).reshape(D, D)
    wqkv8 = np.concatenate([_dr_pack(Wq), _dr_pack(Wk), _dr_pack(Wv)], axis=0)
    com = dict(
        wqkv8=wqkv8,
        wo8=_dr_pack(np.asarray(inputs["Wo"], np.float32)),
        w1b=_w1_pack(np.asarray(inputs["W1"], np.float32)),
        w2b=_dr_pack(np.asarray(inputs["W2"], np.float32),
                     dt=ml_dtypes.bfloat16),
        bo=np.asarray(inputs["bo"], np.float32),
        b1=np.asarray(inputs["b1"], np.float32),
        b2=np.asarray(inputs["b2"], np.float32),
        ln1_g=np.asarray(inputs["ln1_g"], np.float32),
        ln1_b=np.asarray(inputs["ln1_b"], np.float32),
        ln2_g=np.asarray(inputs["ln2_g"], np.float32),
        ln2_b=np.asarray(inputs["ln2_b"], np.float32),
    )
    maps = []
    for c in range(N_CORES):
        m = dict(com)
        m["x"] = x[c * S:(c + 1) * S]
        maps.append(m)
    return maps


def _get_nc():
    if "nc" not in _CACHE:
        _CACHE["nc"] = _build()
    return _CACHE["nc"]


def _run(in_maps):
    from concourse.bass_utils import run_bass_kernel_spmd
    nc = _get_nc()
    res = run_bass_kernel_spmd(nc, in_maps, core_ids=list(range(N_CORES)))
    return res.results


def kernel(**inputs):
    in_maps = _shard_inputs(inputs)
    results = _run(in_maps)
    out = np.concatenate([results[c]["y"] for c in range(N_CORES)], axis=0)
    return out.reshape(B, T, D)


# revision 6
# speedup vs baseline: 1.0075x; 1.0075x over previous
"""Distributed Trainium2 kernel for a dense transformer block (v2).

Sharding (8 cores, one chip):
  - Each core owns S=512 of the 4096 tokens (sequence-parallel LN/FFN/residual)
    and one head-pair (2 of 16 heads) for attention.
  - QKV projections computed locally on own tokens for ALL heads (fp8
    DoubleRow), then ONE fused AllToAll redistributes q/k/v to head-owners.
  - Attention per head-pair over all tokens; softmax denominator folded into
    the AV matmul via a ones-column on V; AV runs fp8 DoubleRow over key-tile
    pairs; the scores->exp->AV chain is software-pipelined so the PE never
    waits on the activation engine.  Normalized outputs return to
    token-owners via a second (small) fp8 AllToAll.
  - Wo runs fp8 DoubleRow; the FFN runs bf16 (fp8 there fails the accuracy
    budget) with weights streamed from DRAM in 512KB chunks on the gpsimd
    DMA queue.
"""

import sys

sys.path.insert(0, "/opt/trn_rl_repo")

import numpy as np

import concourse.bacc as bacc
import concourse.bass as bass
import concourse.tile as tile
from concourse import mybir
from concourse.masks import make_identity

F32 = mybir.dt.float32
BF16 = mybir.dt.bfloat16
FP8 = mybir.dt.float8e4
AF = mybir.ActivationFunctionType
DR = mybir.MatmulPerfMode.DoubleRow

N_CORES = 8
B, T, D, H = 2, 2048, 1024, 16
HD = D // H            # 64
NTOK = B * T           # 4096
S = NTOK // N_CORES    # 512 tokens per core
F = 4 * D              # 4096
EPS = 1e-5
SCALE = float(D) ** -0.5
MASK_VAL = -30000.0
P = 128

KT = D // P            # 8 feature tiles
TT = S // P            # 4 token tiles in the shard
NW = N_CORES           # 8 global 512-token windows
FT = F // P            # 32 ffn-hidden tiles
CP = D // 256          # 4 contraction chunk-pairs over D
FJ = F // 256          # 16 chunk-pairs over F
BLK = 3 * P            # 384 rows per a2a1 block (q,k,v)

_CACHE = {}


def _build(n_chain=1, stub_cc=False):
    nc = bacc.Bacc("TRN2", target_bir_lowering=False, debug=False,
                   num_devices=N_CORES)

    x = nc.dram_tensor("x", [S, D], F32, kind="ExternalInput")
    wqkv8 = nc.dram_tensor("wqkv8", [3 * 512, 2048], FP8, kind="ExternalInput")
    wo8 = nc.dram_tensor("wo8", [512, 2048], FP8, kind="ExternalInput")
    w1b = nc.dram_tensor("w1b", [F // 2, 2 * D], BF16, kind="ExternalInput")
    w2b = nc.dram_tensor("w2b", [F // 2, 2 * D], BF16, kind="ExternalInput")
    bo = nc.dram_tensor("bo", [D], F32, kind="ExternalInput")
    b1 = nc.dram_tensor("b1", [F], F32, kind="ExternalInput")
    b2 = nc.dram_tensor("b2", [D], F32, kind="ExternalInput")
    ln1_g = nc.dram_tensor("ln1_g", [D], F32, kind="ExternalInput")
    ln1_b = nc.dram_tensor("ln1_b", [D], F32, kind="ExternalInput")
    ln2_g = nc.dram_tensor("ln2_g", [D], F32, kind="ExternalInput")
    ln2_b = nc.dram_tensor("ln2_b", [D], F32, kind="ExternalInput")
    y = nc.dram_tensor("y", [S, D], F32, kind="ExternalOutput")
    global _W
    _W = dict(wqkv8=wqkv8, wo8=wo8, w1b=w1b, w2b=w2b, bo=bo, b1=b1, b2=b2,
              ln1_g=ln1_g, ln1_b=ln1_b, ln2_g=ln2_g, ln2_b=ln2_b)

    with tile.TileContext(nc) as tc:
      with tc.tile_pool(name="xch", bufs=1) as xchp, \
           tc.tile_pool(name="gw", bufs=1) as gwp:
        xsets = [[xchp.tile([P, D], F32, tag=f"x{s}_{i}", name=f"x{s}_{i}")
                  for i in range(TT)] for s in range(2)]
        wsb = [gwp.tile([P, 8192], FP8, tag=f"w{pj}", name=f"w{pj}")
               for pj in range(3)]
        for pj in range(3):
            nc.gpsimd.dma_start(
                out=wsb[pj][:].rearrange("p (c n) -> p c n", c=CP),
                in_=wqkv8[pj * 512:(pj + 1) * 512, :].rearrange(
                    "(c p) n -> p c n", p=P))
        wos = [gwp.tile([P, 2048], FP8, tag=f"wos{c}", name=f"wos{c}")
               for c in range(CP)]
        for c in range(CP):
            nc.gpsimd.dma_start(out=wos[c][:], in_=wo8[c * P:(c + 1) * P, :])
        for _ci in range(n_chain):
            _emit_body(nc, tc,
                       x if _ci == 0 else None,
                       y if _ci == n_chain - 1 else None,
                       xsets[_ci % 2], xsets[(_ci + 1) % 2],
                       wsb, wos, _ci, stub_cc)

    nc.compile()
    return nc


def _emit_body(nc, tc, x, y, x_sb, x_out, wsb, wos, ci, stub_cc=False):
    wqkv8, wo8, w1b, w2b = _W["wqkv8"], _W["wo8"], _W["w1b"], _W["w2b"]
    bo, b1, b2 = _W["bo"], _W["b1"], _W["b2"]
    ln1_g, ln1_b = _W["ln1_g"], _W["ln1_b"]
    ln2_g, ln2_b = _W["ln2_g"], _W["ln2_b"]

    with tc.tile_pool(name="dram", bufs=1, space="DRAM") as dram, \
         tc.tile_pool(name="const", bufs=1) as const, \
         tc.tile_pool(name="persist", bufs=1) as persist:

        a2a1_in = dram.tile([NW * 2 * P, S], FP8, tag="a1i", name="a2a1_in")
        a2a1_out = dram.tile([NW * 2 * P, S], FP8, tag="a1o",
                             name="a2a1_out")
        a2av_in = dram.tile([NW * P, S], FP8, tag="avi", name="a2av_in")
        a2av_out = dram.tile([NW * P, S], FP8, tag="avo", name="a2av_out")
        a2a2_in = dram.tile([NW * P, S], FP8, tag="a2i", name="a2a2_in")
        a2a2_out = dram.tile([NW * P, S], FP8, tag="a2o", name="a2a2_out")

        # ---- constants ----
        ident_b = const.tile([P, P], BF16, tag="identb", name="ident_b")
        make_identity(nc, ident_b)

        eps_t = const.tile([P, 1], F32, tag="eps", name="eps_t")
        nc.vector.memset(eps_t[:], EPS)

        # triangular mask for the diagonal 128x128 sub-block:
        # m[r, q] = 0 if r <= q else MASK_VAL
        mask_tri = const.tile([P, P], F32, tag="mtri", name="mask_tri")
        nc.gpsimd.memset(mask_tri[:], 0.0)
        nc.gpsimd.affine_select(
            out=mask_tri[:], in_=mask_tri[:],
            compare_op=mybir.AluOpType.is_ge,
            fill=MASK_VAL, base=0,
            pattern=[[1, P]], channel_multiplier=-1,
        )

        g1_s = const.tile([P, KT], F32, tag="g1", name="g1_s")
        b1l_s = const.tile([P, KT], F32, tag="b1l", name="b1l_s")
        g2_s = const.tile([P, KT], F32, tag="g2", name="g2_s")
        b2l_s = const.tile([P, KT], F32, tag="b2l", name="b2l_s")
        nc.scalar.dma_start(out=g1_s[:],
                            in_=ln1_g.ap().rearrange("(k p) -> p k", p=P))
        nc.scalar.dma_start(out=b1l_s[:],
                            in_=ln1_b.ap().rearrange("(k p) -> p k", p=P))
        nc.scalar.dma_start(out=g2_s[:],
                            in_=ln2_g.ap().rearrange("(k p) -> p k", p=P))
        nc.scalar.dma_start(out=b2l_s[:],
                            in_=ln2_b.ap().rearrange("(k p) -> p k", p=P))

        b1_s = const.tile([P, FT], F32, tag="b1s", name="b1_s")
        nc.scalar.dma_start(out=b1_s[:],
                            in_=b1.ap().rearrange("(k p) -> p k", p=P))

        bo_bc = const.tile([P, D], F32, tag="bo_bc", name="bo_bc")
        b2_bc = const.tile([P, D], F32, tag="b2_bc", name="b2_bc")
        nc.scalar.dma_start(out=bo_bc[:], in_=bo.ap().partition_broadcast(P))
        nc.scalar.dma_start(out=b2_bc[:], in_=b2.ap().partition_broadcast(P))

        # persistent: r1; x shard tiles passed in (SBUF-resident chain)
        if x is not None:
            for i in range(TT):
                nc.sync.dma_start(out=x_sb[i][:],
                                  in_=x[i * P:(i + 1) * P, :])
        r1 = [persist.tile([P, D], F32, tag=f"r1_{i}", name=f"r1_{i}")
              for i in range(TT)]

        def layernorm_tiles(src_tiles, pool, out_name):
            """LN over the free axis -> normalized bf16 [t,d] tiles
            (gamma/beta folded in later at transpose-evict)."""
            out = []
            with tc.tile_pool(name=f"ln_{out_name}", bufs=2) as lnp:
                for i, xt in enumerate(src_tiles):
                    st = lnp.tile([P, 2, 6], F32, tag="stats", name="st")
                    xr = xt[:].rearrange("p (s f) -> p s f", s=2)
                    for sg in range(2):
                        nc.vector.bn_stats(out=st[:, sg, :], in_=xr[:, sg, :])
                    mv = lnp.tile([P, 2], F32, tag="mv", name="mv")
                    nc.vector.bn_aggr(out=mv[:], in_=st[:])
                    rstd = lnp.tile([P, 1], F32, tag="rstd", name="rstd")
                    nc.scalar.activation(out=rstd[:], in_=mv[:, 1:2],
                                         func=AF.Sqrt, bias=eps_t[:],
                                         scale=1.0)
                    nc.vector.reciprocal(out=rstd[:], in_=rstd[:])
                    o = pool.tile([P, D], BF16, tag=f"{out_name}{i}",
                                  name=f"{out_name}{i}")
                    nc.vector.tensor_scalar(
                        out=o[:], in0=xt[:],
                        scalar1=mv[:, 0:1], scalar2=rstd[:],
                        op0=mybir.AluOpType.subtract,
                        op1=mybir.AluOpType.mult,
                    )
                    out.append(o)
            return out

        def pair_view(t, width):
            """[128, 2*width] tile -> [128, 2, width] DoubleRow view."""
            return t[:].rearrange("p (two n) -> p two n", two=2)

        # ================= QKV phase =================
        with tc.tile_pool(name="qkvsb", bufs=1) as qkvsb:
            # LN1 + transpose, folding gamma/beta, writing fp8 pair tiles
            h8 = [qkvsb.tile([P, 1024], FP8, tag=f"h8_{c}", name=f"h8_{c}")
                  for c in range(CP)]
            with tc.tile_pool(name="xlnp", bufs=1) as xlnp:
                xln = layernorm_tiles(x_sb, xlnp, "xln")
                with tc.tile_pool(name="tr1p", bufs=4, space="PSUM") as trpp:
                    for i in range(TT):
                        for kt in range(KT):
                            pt = trpp.tile([P, P], BF16, tag="tr", name="pt")
                            nc.tensor.transpose(
                                pt[:], xln[i][:, kt * P:(kt + 1) * P],
                                ident_b[:])
                            nc.vector.tensor_scalar(
                                out=h8[kt // 2][:, (kt % 2) * 512 + i * P:
                                                (kt % 2) * 512 + (i + 1) * P],
                                in0=pt[:],
                                scalar1=g1_s[:, kt:kt + 1],
                                scalar2=b1l_s[:, kt:kt + 1],
                                op0=mybir.AluOpType.mult,
                                op1=mybir.AluOpType.add,
                            )

            def wqkv_view(pj, c, ob):
                v = wsb[pj][:].rearrange("p (c two n) -> p c two n",
                                         c=CP, two=2)
                return v[:, c, :, ob * P:(ob + 1) * P]

            with tc.tile_pool(name="qkvst", bufs=3) as qst, \
                 tc.tile_pool(name="qkvp", bufs=2, space="PSUM") as qkvp, \
                 tc.tile_pool(name="vtp", bufs=2, space="PSUM") as vtp:
                pend_v = []

                def emit_vtr():
                    stv, ob = pend_v.pop(0)
                    for j4 in range(4):
                        pv = vtp.tile([P, P], BF16, tag="pv", name="pv")
                        nc.tensor.transpose(pv[:], stv[:, j4 * P:(j4 + 1) * P],
                                            ident_b[:])
                        v8 = qst.tile([P, P], FP8, tag="v8", name="v8")
                        nc.vector.tensor_copy(v8[:], pv[:])
                        nc.sync.dma_start(
                            out=a2av_in[ob * P + 32 * j4:
                                        ob * P + 32 * (j4 + 1),
                                        :].rearrange("r (s f) -> (r s) f",
                                                     s=4),
                            in_=v8[:])

                for ob in range(NW):
                    pss = [qkvp.tile([P, 512], F32, tag=f"pp{pj}",
                                     name=f"pp{pj}") for pj in range(3)]
                    for c in range(CP):
                        for pj in range(3):
                            nc.tensor.matmul(
                                pss[pj][:],
                                wqkv_view(pj, c, ob),
                                pair_view(h8[c], 512),
                                start=(c == 0), stop=(c == CP - 1),
                                perf_mode=DR)
                    st = qst.tile([P, 1024], FP8, tag="st", name="st")
                    nc.scalar.copy(st[:, 0:512], pss[0][:])
                    nc.scalar.copy(st[:, 512:1024], pss[1][:])
                    nc.sync.dma_start(
                        out=a2a1_in[ob * 2 * P:(ob + 1) * 2 * P,
                                    :].rearrange(
                            "(blk p) c -> p blk c", p=P),
                        in_=st[:].rearrange("p (blk c) -> p blk c", blk=2))
                    stv = qst.tile([P, 512], BF16, tag="stv", name="stv")
                    nc.vector.tensor_copy(stv[:], pss[2][:])
                    pend_v.append((stv, ob))
                    if len(pend_v) > 1:
                        emit_vtr()
                while pend_v:
                    emit_vtr()

        if stub_cc:
            nc.sync.dma_start(out=a2a1_out[:, :], in_=a2a1_in[:, :])
            nc.sync.dma_start(out=a2av_out[:, :], in_=a2av_in[:, :])
        else:
            nc.gpsimd.collective_compute(
                "AllToAll", mybir.AluOpType.bypass,
                replica_groups=[list(range(N_CORES))],
                ins=[a2a1_in.opt()], outs=[a2a1_out.opt()],
            )
            nc.gpsimd.collective_compute(
                "AllToAll", mybir.AluOpType.bypass,
                replica_groups=[list(range(N_CORES))],
                ins=[a2av_in.opt()], outs=[a2av_out.opt()],
            )

        # ============ attention super-phase ============
        with tc.tile_pool(name="attnsb", bufs=1) as attnsb:
            # x + bo (residual base), done while collective runs
            for i in range(TT):
                nc.vector.tensor_add(out=x_sb[i][:], in0=x_sb[i][:],
                                     in1=bo_bc[:])

            # qkv_w[w]: [128, 1024] = qT | kT for window w
            qkv_w = [attnsb.tile([P, 2 * 512], FP8, tag=f"qkv{w}",
                                 name=f"qkv{w}") for w in range(NW)]
            # v_aug per head per key-tile pair: [128 keys, 2, 96]
            # (64 v cols + ones col + zero pad; DR stationary width must be
            # a multiple of 32)
            vgA = [attnsb.tile([P, 192], FP8, tag=f"vgA{i}", name=f"vgA{i}")
                   for i in range(16)]
            vgB = [attnsb.tile([P, 192], FP8, tag=f"vgB{i}", name=f"vgB{i}")
                   for i in range(16)]
            for i in range(16):
                for vg in (vgA[i], vgB[i]):
                    nc.vector.memset(vg[:], 0.0)
                    nc.vector.memset(vg[:, HD:HD + 1], 1.0)
                    nc.vector.memset(vg[:, 96 + HD:96 + HD + 1], 1.0)

            # prefetch all window loads up front; v goes straight into
            # the v_aug pair tiles (token-major already)
            for w in range(NW):
                nc.sync.dma_start(
                    out=qkv_w[w][:].rearrange("p (blk c) -> p blk c",
                                              blk=2),
                    in_=a2a1_out[w * 2 * P:(w + 1) * 2 * P, :].rearrange(
                        "(blk p) c -> p blk c", p=P))
                for si in range(4):
                    vsrc = a2av_out[w * P + 32 * si:
                                    w * P + 32 * (si + 1),
                                    :].rearrange("r (s f) -> (r s) f", s=4)
                    pi = w * 2 + si // 2
                    col = (si % 2) * 96
                    nc.gpsimd.dma_start(out=vgA[pi][:, col:col + HD],
                                        in_=vsrc[:, 0:HD])
                    nc.gpsimd.dma_start(out=vgB[pi][:, col:col + HD],
                                        in_=vsrc[:, HD:P])

            o_sbs = [attnsb.tile([65, 512], BF16, tag=f"osb{i}",
                                 name=f"osb{i}") for i in range(16)]
            with tc.tile_pool(name="scp", bufs=3, space="PSUM") as scp, \
                 tc.tile_pool(name="ptp", bufs=6) as ptp, \
                 tc.tile_pool(name="lop", bufs=1, space="PSUM") as lop, \
                 tc.tile_pool(name="nrm2", bufs=3) as nrm2:

                def attn_window(b, tcl):
                    tch = b * 4 + tcl
                    n_s = 4 * (tcl + 1)
                    np2 = n_s // 2
                    qT = qkv_w[tch]
                    o_ps = {}
                    for hh in range(2):
                        o_ps[hh] = lop.tile([96, 512], F32, tag=f"ops{hh}",
                                            name=f"o_ps{hh}")
                    pend = []

                    def emit_sc(j, hh):
                        rows = slice(hh * HD, (hh + 1) * HD)
                        sc = scp.tile([P, 1024], F32, tag="sc", name="sc")
                        diag = []
                        for half in range(2):
                            si = 2 * j + half
                            kw = b * 4 + si // 4
                            kc = 512 + (si % 4) * P
                            nc.tensor.matmul(
                                sc[:, half * 512:(half + 1) * 512],
                                qkv_w[kw][rows, kc:kc + P],
                                qT[rows, 0:512],
                                start=True, stop=True,
                                tile_position=(hh * HD, 0))
                            if si // 4 == tcl:
                                diag.append((half, (si % 4) * P))
                        p = ptp.tile([P, 1024], FP8, tag="p", name="p")
                        if not diag:
                            nc.scalar.activation(
                                out=p[:], in_=sc[:],
                                func=AF.Exp, scale=SCALE)
                        else:
                            for half, c0 in diag:
                                base = half * 512
                                nc.vector.tensor_add(
                                    out=sc[:, base + c0:base + c0 + P],
                                    in0=sc[:, base + c0:base + c0 + P],
                                    in1=mask_tri[:])
                                if c0:
                                    nc.vector.memset(
                                        p[:, base:base + c0], 0.0)
                                nc.scalar.activation(
                                    out=p[:, base + c0:base + 512],
                                    in_=sc[:, base + c0:base + 512],
                                    func=AF.Exp, scale=SCALE)
                        pend.append((p, j, hh))

                    def emit_av():
                        p, j, hh = pend.pop(0)
                        vg = vgA if hh == 0 else vgB
                        nc.tensor.matmul(
                            o_ps[hh][:], pair_view(vg[b * 8 + j], 96),
                            pair_view(p, 512),
                            start=(j == 0), stop=(j == np2 - 1),
                            perf_mode=DR)

                    for j in range(np2):
                        emit_sc(j, 0)
                        emit_sc(j, 1)
                        while len(pend) > 2:
                            emit_av()
                    while pend:
                        emit_av()
                    for hh in range(2):
                        osb = o_sbs[tch * 2 + hh]
                        nc.vector.tensor_copy(osb[:], o_ps[hh][0:65, :])

                def normalize_entry(idx):
                    osb = o_sbs[idx]
                    lf = nrm2.tile([65, 512], F32, tag="lf", name="lf")
                    nc.vector.tensor_copy(lf[64:65, :], osb[64:65, :])
                    l0 = nrm2.tile([1, 512], F32, tag="l0", name="l0")
                    nc.sync.dma_start(out=l0[:], in_=lf[64:65, :])
                    lr = nrm2.tile([1, 512], F32, tag="lr", name="lr")
                    nc.vector.reciprocal_approx_fast(out=lr[:], in_=l0[:])
                    lbc = nrm2.tile([HD, 512], F32, tag="lbc", name="lbc")
                    nc.gpsimd.partition_broadcast(lbc[:], lr[0:1, :])
                    o_n = nrm2.tile([HD, 512], FP8, tag="on", name="o_n")
                    nc.vector.tensor_mul(out=o_n[:], in0=osb[0:HD, :],
                                         in1=lbc[:])
                    nc.sync.dma_start(
                        out=a2a2_in[idx * HD:(idx + 1) * HD, :],
                        in_=o_n[:])

                for b in range(B):
                    for tcl in range(4):
                        attn_window(b, tcl)
                for idx in range(16):
                    normalize_entry(idx)

            if stub_cc:
                nc.sync.dma_start(out=a2a2_out[:, :], in_=a2a2_in[:, :])
            else:
                nc.gpsimd.collective_compute(
                    "AllToAll", mybir.AluOpType.bypass,
                    replica_groups=[list(range(N_CORES))],
                    ins=[a2a2_in.opt()], outs=[a2a2_out.opt()],
                )

            # ---- Wo projection (DoubleRow) + residual ----
            with tc.tile_pool(name="wosb", bufs=1) as wosb, \
                 tc.tile_pool(name="wop", bufs=1, space="PSUM") as wop:
                o8 = [wosb.tile([P, 1024], FP8, tag=f"o8_{c}", name=f"o8_{c}")
                      for c in range(CP)]
                for c in range(CP):
                    nc.sync.dma_start(
                        out=o8[c][:].rearrange("p (two c) -> p two c", two=2),
                        in_=a2a2_out[c * 256:(c + 1) * 256, :].rearrange(
                            "(two p) c -> p two c", p=P))
                pso = [wop.tile([P, 512], F32, tag=f"wo{i}", name=f"wo{i}")
                       for i in range(8)]
                for c in range(CP):
                    for tt in range(TT):
                        lhs = pair_view(o8[c], 512)[:, :, tt * P:(tt + 1) * P]
                        for dc in range(2):
                            nc.tensor.matmul(
                                pso[tt * 2 + dc][:], lhs,
                                pair_view(wos[c], 1024)[:, :,
                                                        dc * 512:
                                                        (dc + 1) * 512],
                                start=(c == 0), stop=(c == CP - 1),
                                perf_mode=DR)
                for tt in range(TT):
                    for dc in range(2):
                        sl = slice(dc * 512, (dc + 1) * 512)
                        nc.vector.tensor_add(out=r1[tt][:, sl],
                                             in0=pso[tt * 2 + dc][:],
                                             in1=x_sb[tt][:, sl])

            # ---- LN2 + transpose (bf16, kt-major) ----
            h2T = [attnsb.tile([P, 512], BF16, tag=f"h2T{k}",
                               name=f"h2T{k}") for k in range(KT)]
            with tc.tile_pool(name="h2p", bufs=1) as h2p:
                h2 = layernorm_tiles(r1, h2p, "h2")
                with tc.tile_pool(name="tr2p", bufs=4, space="PSUM") as tr2p:
                    for i in range(TT):
                        for kt in range(KT):
                            pt2 = tr2p.tile([P, P], BF16, tag="tr2",
                                            name="pt2")
                            nc.tensor.transpose(
                                pt2[:], h2[i][:, kt * P:(kt + 1) * P],
                                ident_b[:])
                            nc.vector.tensor_scalar(
                                out=h2T[kt][:, i * P:(i + 1) * P],
                                in0=pt2[:],
                                scalar1=g2_s[:, kt:kt + 1],
                                scalar2=b2l_s[:, kt:kt + 1],
                                op0=mybir.AluOpType.mult,
                                op1=mybir.AluOpType.add,
                            )

            # r1 + b2 (residual base for ffn out)
            for i in range(TT):
                nc.vector.tensor_add(out=r1[i][:], in0=r1[i][:],
                                     in1=b2_bc[:])

            # ---- FFN (bf16, streamed weights) ----
            ff1 = [attnsb.tile([P, 512], BF16, tag=f"ff1_{k}",
                               name=f"ff1_{k}") for k in range(FT)]
            with tc.tile_pool(name="w1st", bufs=4) as w1st, \
                 tc.tile_pool(name="ff1pp", bufs=4, space="PSUM") as ff1pp:
                for j in range(FJ):
                    w1t = w1st.tile([P, 2048], BF16, tag="w1t", name="w1t")
                    nc.gpsimd.dma_start(out=w1t[:],
                                        in_=w1b[j * P:(j + 1) * P, :])
                    for half in range(2):
                        ft = 2 * j + half
                        ps = ff1pp.tile([P, 512], F32, tag="ff1", name="ps")
                        for kt in range(KT):
                            nc.tensor.matmul(
                                ps[:],
                                w1t[:, half * 1024 + kt * P:
                                    half * 1024 + (kt + 1) * P],
                                h2T[kt][:],
                                start=(kt == 0), stop=(kt == KT - 1))
                        nc.scalar.activation(
                            out=ff1[ft][:],
                            in_=ps[:], func=AF.Relu, bias=b1_s[:, ft:ft + 1])

            with tc.tile_pool(name="w2st", bufs=4) as w2st, \
                 tc.tile_pool(name="outp", bufs=4) as outp, \
                 tc.tile_pool(name="ff2p", bufs=1, space="PSUM") as ff2p:
                ps2 = [ff2p.tile([P, 512], F32, tag=f"ff2_{i}",
                                 name=f"ff2_{i}") for i in range(8)]
                w2ts = []
                for j in range(2):
                    w2t = w2st.tile([P, 2048], BF16, tag="w2t", name="w2t")
                    nc.scalar.dma_start(out=w2t[:],
                                        in_=w2b[j * P:(j + 1) * P, :])
                    w2ts.append(w2t)
                for j in range(FJ):
                    if j < 2:
                        w2t = w2ts[j]
                    else:
                        w2t = w2st.tile([P, 2048], BF16, tag="w2t",
                                        name="w2t")
                        nc.gpsimd.dma_start(out=w2t[:],
                                            in_=w2b[j * P:(j + 1) * P, :])
                    for half in range(2):
                        kt = 2 * j + half
                        for tt in range(TT):
                            lhs = ff1[kt][:, tt * P:(tt + 1) * P]
                            for dc in range(2):
                                nc.tensor.matmul(
                                    ps2[tt * 2 + dc][:], lhs,
                                    w2t[:, half * 1024 + dc * 512:
                                        half * 1024 + (dc + 1) * 512],
                                    start=(kt == 0), stop=(kt == FT - 1))
                for tt in range(TT):
                    for dc in range(2):
                        sl = slice(dc * 512, (dc + 1) * 512)
                        nc.vector.tensor_add(out=x_out[tt][:, sl],
                                             in0=ps2[tt * 2 + dc][:],
                                             in1=r1[tt][:, sl])
                    if y is not None:
                        nc.sync.dma_start(out=y[tt * P:(tt + 1) * P, :],
                                          in_=x_out[tt][:])


def _dr_pack(w, dt=None):
    """[K, N] -> DoubleRow pair layout [K/2, 2N]: row c*128+p,
    col i*N+n  holds  w[c*256 + i*128 + p, n]."""
    import ml_dtypes
    if dt is None:
        dt = ml_dtypes.float8_e4m3fn
    K, N = w.shape
    nch = K // 256
    t = w.reshape(nch, 2, 128, N).transpose(0, 2, 1, 3).reshape(K // 2, 2 * N)
    return np.ascontiguousarray(t.astype(dt))


def _w1_pack(w1):
    """[D, F] -> streamed lhsT pair tiles: row j*128+p,
    col half*1024 + kt*128 + c  holds  W1[kt*128+p, (2j+half)*128+c]."""
    import ml_dtypes
    t = w1.reshape(KT, P, FJ, 2, P).transpose(2, 1, 3, 0, 4).reshape(
        F // 2, 2 * D)
    return np.ascontiguousarray(t.astype(ml_dtypes.bfloat16))


def _shard_inputs(inputs):
    import ml_dtypes
    x = np.ascontiguousarray(
        np.asarray(inputs["x"], np.float32).reshape(NTOK, D))
    Wq = np.asarray(inputs["Wq"], np.float32).transpose(1, 0, 2).reshape(D, D)
    Wk = np.asarray(inputs["Wk"], np.float32).transpose(1, 0, 2).reshape(D, D)
    Wv = np.asarray(inputs["Wv"], np.float32).transpose(1, 0, 2).reshape(D, D)
    wqkv8 = np.concatenate([_dr_pack(Wq), _dr_pack(Wk), _dr_pack(Wv)], axis=0)
    com = dict(
        wqkv8=wqkv8,
        wo8=_dr_pack(np.asarray(inputs["Wo"], np.float32)),
        w1b=_w1_pack(np.asarray(inputs["W1"], np.float32)),
        w2b=_dr_pack(np.asarray(inputs["W2"], np.float32),
                     dt=ml_dtypes.bfloat16),
        bo=np.asarray(inputs["bo"], np.float32),
        b1=np.asarray(inputs["b1"], np.float32),
        b2=np.asarray(inputs["b2"], np.float32),
        ln1_g=np.asarray(inputs["ln1_g"], np.float32),
        ln1_b=np.asarray(inputs["ln1_b"], np.float32),
        ln2_g=np.asarray(inputs["ln2_g"], np.float32),
        ln2_b=np.asarray(inputs["ln2_b"], np.float32),
    )
    maps = []
    for c in range(N_CORES):
        m = dict(com)
        m["x"] = x[c * S:(c + 1) * S]
        maps.append(m)
    return maps


def _get_nc():
    if "nc" not in _CACHE:
        _CACHE["nc"] = _build()
    return _CACHE["nc"]


def _run(in_maps):
    from concourse.bass_utils import run_bass_kernel_spmd
    nc = _get_nc()
    res = run_bass_kernel_spmd(nc, in_maps, core_ids=list(range(N_CORES)))
    return res.results


def kernel(**inputs):
    in_maps = _shard_inputs(inputs)
    results = _run(in_maps)
    out = np.concatenate([results[c]["y"] for c in range(N_CORES)], axis=0)
    return out.reshape(B, T, D)


# revision 9
# speedup vs baseline: 1.0123x; 1.0048x over previous
"""Distributed Trainium2 kernel for a dense transformer block (v2).

Sharding (8 cores, one chip):
  - Each core owns S=512 of the 4096 tokens (sequence-parallel LN/FFN/residual)
    and one head-pair (2 of 16 heads) for attention.
  - QKV projections computed locally on own tokens for ALL heads (fp8
    DoubleRow), then ONE fused AllToAll redistributes q/k/v to head-owners.
  - Attention per head-pair over all tokens; softmax denominator folded into
    the AV matmul via a ones-column on V; AV runs fp8 DoubleRow over key-tile
    pairs; the scores->exp->AV chain is software-pipelined so the PE never
    waits on the activation engine.  Normalized outputs return to
    token-owners via a second (small) fp8 AllToAll.
  - Wo runs fp8 DoubleRow; the FFN runs bf16 (fp8 there fails the accuracy
    budget) with weights streamed from DRAM in 512KB chunks on the gpsimd
    DMA queue.
"""

import sys

sys.path.insert(0, "/opt/trn_rl_repo")

import numpy as np

import concourse.bacc as bacc
import concourse.bass as bass
import concourse.tile as tile
from concourse import mybir
from concourse.masks import make_identity

F32 = mybir.dt.float32
BF16 = mybir.dt.bfloat16
FP8 = mybir.dt.float8e4
AF = mybir.ActivationFunctionType
DR = mybir.MatmulPerfMode.DoubleRow

N_CORES = 8
B, T, D, H = 2, 2048, 1024, 16
HD = D // H            # 64
NTOK = B * T           # 4096
S = NTOK // N_CORES    # 512 tokens per core
F = 4 * D              # 4096
EPS = 1e-5
SCALE = float(D) ** -0.5
MASK_VAL = -30000.0
P = 128

KT = D // P            # 8 feature tiles
TT = S // P            # 4 token tiles in the shard
NW = N_CORES           # 8 global 512-token windows
FT = F // P            # 32 ffn-hidden tiles
CP = D // 256          # 4 contraction chunk-pairs over D
FJ = F // 256          # 16 chunk-pairs over F
BLK = 3 * P            # 384 rows per a2a1 block (q,k,v)

_CACHE = {}


def _build(n_chain=1, stub_cc=False):
    nc = bacc.Bacc("TRN2", target_bir_lowering=False, debug=False,
                   num_devices=N_CORES)

    x = nc.dram_tensor("x", [S, D], F32, kind="ExternalInput")
    wqkv8 = nc.dram_tensor("wqkv8", [3 * 512, 2048], FP8, kind="ExternalInput")
    wo8 = nc.dram_tensor("wo8", [512, 2048], FP8, kind="ExternalInput")
    w1b = nc.dram_tensor("w1b", [F // 2, 2 * D], BF16, kind="ExternalInput")
    w2b = nc.dram_tensor("w2b", [F // 2, 2 * D], BF16, kind="ExternalInput")
    bo = nc.dram_tensor("bo", [D], F32, kind="ExternalInput")
    b1 = nc.dram_tensor("b1", [F], F32, kind="ExternalInput")
    b2 = nc.dram_tensor("b2", [D], F32, kind="ExternalInput")
    ln1_g = nc.dram_tensor("ln1_g", [D], F32, kind="ExternalInput")
    ln1_b = nc.dram_tensor("ln1_b", [D], F32, kind="ExternalInput")
    ln2_g = nc.dram_tensor("ln2_g", [D], F32, kind="ExternalInput")
    ln2_b = nc.dram_tensor("ln2_b", [D], F32, kind="ExternalInput")
    y = nc.dram_tensor("y", [S, D], F32, kind="ExternalOutput")
    global _W
    _W = dict(wqkv8=wqkv8, wo8=wo8, w1b=w1b, w2b=w2b, bo=bo, b1=b1, b2=b2,
              ln1_g=ln1_g, ln1_b=ln1_b, ln2_g=ln2_g, ln2_b=ln2_b)

    with tile.TileContext(nc) as tc:
      with tc.tile_pool(name="xch", bufs=1) as xchp, \
           tc.tile_pool(name="gw", bufs=1) as gwp:
        xsets = [[xchp.tile([P, D], F32, tag=f"x{s}_{i}", name=f"x{s}_{i}")
                  for i in range(TT)] for s in range(2)]
        wsb = [gwp.tile([P, 8192], FP8, tag=f"w{pj}", name=f"w{pj}")
               for pj in range(3)]
        for pj in range(3):
            nc.gpsimd.dma_start(
                out=wsb[pj][:].rearrange("p (c n) -> p c n", c=CP),
                in_=wqkv8[pj * 512:(pj + 1) * 512, :].rearrange(
                    "(c p) n -> p c n", p=P))
        wos = [gwp.tile([P, 2048], FP8, tag=f"wos{c}", name=f"wos{c}")
               for c in range(CP)]
        for c in range(CP):
            nc.gpsimd.dma_start(out=wos[c][:], in_=wo8[c * P:(c + 1) * P, :])
        for _ci in range(n_chain):
            _emit_body(nc, tc,
                       x if _ci == 0 else None,
                       y if _ci == n_chain - 1 else None,
                       xsets[_ci % 2], xsets[(_ci + 1) % 2],
                       wsb, wos, _ci, stub_cc)

    nc.compile()
    return nc


def _emit_body(nc, tc, x, y, x_sb, x_out, wsb, wos, ci, stub_cc=False):
    wqkv8, wo8, w1b, w2b = _W["wqkv8"], _W["wo8"], _W["w1b"], _W["w2b"]
    bo, b1, b2 = _W["bo"], _W["b1"], _W["b2"]
    ln1_g, ln1_b = _W["ln1_g"], _W["ln1_b"]
    ln2_g, ln2_b = _W["ln2_g"], _W["ln2_b"]

    with tc.tile_pool(name="dram", bufs=1, space="DRAM") as dram, \
         tc.tile_pool(name="const", bufs=1) as const, \
         tc.tile_pool(name="persist", bufs=1) as persist:

        a2a1_in = dram.tile([NW * 2 * P, S], FP8, tag="a1i", name="a2a1_in")
        a2a1_out = dram.tile([NW * 2 * P, S], FP8, tag="a1o",
                             name="a2a1_out")
        a2av_in = dram.tile([NW * P, S], FP8, tag="avi", name="a2av_in")
        a2av_out = dram.tile([NW * P, S], FP8, tag="avo", name="a2av_out")
        a2a2_in = dram.tile([NW * P, S], FP8, tag="a2i", name="a2a2_in")
        a2a2_out = dram.tile([NW * P, S], FP8, tag="a2o", name="a2a2_out")

        # ---- constants ----
        ident_b = const.tile([P, P], BF16, tag="identb", name="ident_b")
        make_identity(nc, ident_b)

        eps_t = const.tile([P, 1], F32, tag="eps", name="eps_t")
        nc.vector.memset(eps_t[:], EPS)

        # triangular mask for the diagonal 128x128 sub-block:
        # m[r, q] = 0 if r <= q else MASK_VAL
        mask_tri = const.tile([P, P], F32, tag="mtri", name="mask_tri")
        nc.gpsimd.memset(mask_tri[:], 0.0)
        nc.gpsimd.affine_select(
            out=mask_tri[:], in_=mask_tri[:],
            compare_op=mybir.AluOpType.is_ge,
            fill=MASK_VAL, base=0,
            pattern=[[1, P]], channel_multiplier=-1,
        )

        g1_s = const.tile([P, KT], F32, tag="g1", name="g1_s")
        b1l_s = const.tile([P, KT], F32, tag="b1l", name="b1l_s")
        g2_s = const.tile([P, KT], F32, tag="g2", name="g2_s")
        b2l_s = const.tile([P, KT], F32, tag="b2l", name="b2l_s")
        nc.scalar.dma_start(out=g1_s[:],
                            in_=ln1_g.ap().rearrange("(k p) -> p k", p=P))
        nc.scalar.dma_start(out=b1l_s[:],
                            in_=ln1_b.ap().rearrange("(k p) -> p k", p=P))
        nc.scalar.dma_start(out=g2_s[:],
                            in_=ln2_g.ap().rearrange("(k p) -> p k", p=P))
        nc.scalar.dma_start(out=b2l_s[:],
                            in_=ln2_b.ap().rearrange("(k p) -> p k", p=P))

        b1_s = const.tile([P, FT], F32, tag="b1s", name="b1_s")
        nc.scalar.dma_start(out=b1_s[:],
                            in_=b1.ap().rearrange("(k p) -> p k", p=P))

        bo_bc = const.tile([P, D], F32, tag="bo_bc", name="bo_bc")
        b2_bc = const.tile([P, D], F32, tag="b2_bc", name="b2_bc")
        nc.scalar.dma_start(out=bo_bc[:], in_=bo.ap().partition_broadcast(P))
        nc.scalar.dma_start(out=b2_bc[:], in_=b2.ap().partition_broadcast(P))

        # persistent: r1; x shard tiles passed in (SBUF-resident chain)
        if x is not None:
            for i in range(TT):
                nc.sync.dma_start(out=x_sb[i][:],
                                  in_=x[i * P:(i + 1) * P, :])
        r1 = [persist.tile([P, D], F32, tag=f"r1_{i}", name=f"r1_{i}")
              for i in range(TT)]

        def layernorm_tiles(src_tiles, pool, out_name):
            """LN over the free axis -> normalized bf16 [t,d] tiles
            (gamma/beta folded in later at transpose-evict)."""
            out = []
            with tc.tile_pool(name=f"ln_{out_name}", bufs=2) as lnp:
                for i, xt in enumerate(src_tiles):
                    st = lnp.tile([P, 2, 6], F32, tag="stats", name="st")
                    xr = xt[:].rearrange("p (s f) -> p s f", s=2)
                    for sg in range(2):
                        nc.vector.bn_stats(out=st[:, sg, :], in_=xr[:, sg, :])
                    mv = lnp.tile([P, 2], F32, tag="mv", name="mv")
                    nc.vector.bn_aggr(out=mv[:], in_=st[:])
                    rstd = lnp.tile([P, 1], F32, tag="rstd", name="rstd")
                    nc.scalar.activation(out=rstd[:], in_=mv[:, 1:2],
                                         func=AF.Sqrt, bias=eps_t[:],
                                         scale=1.0)
                    nc.vector.reciprocal(out=rstd[:], in_=rstd[:])
                    o = pool.tile([P, D], BF16, tag=f"{out_name}{i}",
                                  name=f"{out_name}{i}")
                    nc.vector.tensor_scalar(
                        out=o[:], in0=xt[:],
                        scalar1=mv[:, 0:1], scalar2=rstd[:],
                        op0=mybir.AluOpType.subtract,
                        op1=mybir.AluOpType.mult,
                    )
                    out.append(o)
            return out

        def pair_view(t, width):
            """[128, 2*width] tile -> [128, 2, width] DoubleRow view."""
            return t[:].rearrange("p (two n) -> p two n", two=2)

        # ================= QKV phase =================
        with tc.tile_pool(name="qkvsb", bufs=1) as qkvsb:
            # LN1 + transpose, folding gamma/beta, writing fp8 pair tiles
            h8 = [qkvsb.tile([P, 1024], FP8, tag=f"h8_{c}", name=f"h8_{c}")
                  for c in range(CP)]
            with tc.tile_pool(name="xlnp", bufs=1) as xlnp:
                xln = layernorm_tiles(x_sb, xlnp, "xln")
                with tc.tile_pool(name="tr1p", bufs=4, space="PSUM") as trpp:
                    for i in range(TT):
                        for kt in range(KT):
                            pt = trpp.tile([P, P], BF16, tag="tr", name="pt")
                            nc.tensor.transpose(
                                pt[:], xln[i][:, kt * P:(kt + 1) * P],
                                ident_b[:])
                            nc.vector.tensor_scalar(
                                out=h8[kt // 2][:, (kt % 2) * 512 + i * P:
                                                (kt % 2) * 512 + (i + 1) * P],
                                in0=pt[:],
                                scalar1=g1_s[:, kt:kt + 1],
                                scalar2=b1l_s[:, kt:kt + 1],
                                op0=mybir.AluOpType.mult,
                                op1=mybir.AluOpType.add,
                            )

            def wqkv_view(pj, c, ob):
                v = wsb[pj][:].rearrange("p (c two n) -> p c two n",
                                         c=CP, two=2)
                return v[:, c, :, ob * P:(ob + 1) * P]

            with tc.tile_pool(name="qkvst", bufs=3) as qst, \
                 tc.tile_pool(name="qkvp", bufs=2, space="PSUM") as qkvp, \
                 tc.tile_pool(name="vtp", bufs=2, space="PSUM") as vtp:
                pend_v = []

                def emit_vtr():
                    stv, ob = pend_v.pop(0)
                    for j4 in range(4):
                        pv = vtp.tile([P, P], BF16, tag="pv", name="pv")
                        nc.tensor.transpose(pv[:], stv[:, j4 * P:(j4 + 1) * P],
                                            ident_b[:])
                        v8 = qst.tile([P, P], FP8, tag="v8", name="v8")
                        nc.vector.tensor_copy(v8[:], pv[:])
                        nc.sync.dma_start(
                            out=a2av_in[ob * P + 32 * j4:
                                        ob * P + 32 * (j4 + 1),
                                        :].rearrange("r (s f) -> (r s) f",
                                                     s=4),
                            in_=v8[:])

                for ob in range(NW):
                    pss = [qkvp.tile([P, 512], F32, tag=f"pp{pj}",
                                     name=f"pp{pj}") for pj in range(3)]
                    for c in range(CP):
                        for pj in range(3):
                            nc.tensor.matmul(
                                pss[pj][:],
                                wqkv_view(pj, c, ob),
                                pair_view(h8[c], 512),
                                start=(c == 0), stop=(c == CP - 1),
                                perf_mode=DR)
                    st = qst.tile([P, 1024], FP8, tag="st", name="st")
                    nc.scalar.copy(st[:, 0:512], pss[0][:])
                    nc.scalar.copy(st[:, 512:1024], pss[1][:])
                    nc.sync.dma_start(
                        out=a2a1_in[ob * 2 * P:(ob + 1) * 2 * P,
                                    :].rearrange(
                            "(blk p) c -> p blk c", p=P),
                        in_=st[:].rearrange("p (blk c) -> p blk c", blk=2))
                    stv = qst.tile([P, 512], BF16, tag="stv", name="stv")
                    nc.vector.tensor_copy(stv[:], pss[2][:])
                    pend_v.append((stv, ob))
                    if len(pend_v) > 1:
                        emit_vtr()
                while pend_v:
                    emit_vtr()

        if stub_cc:
            nc.sync.dma_start(out=a2a1_out[:, :], in_=a2a1_in[:, :])
            nc.sync.dma_start(out=a2av_out[:, :], in_=a2av_in[:, :])
        else:
            nc.gpsimd.collective_compute(
                "AllToAll", mybir.AluOpType.bypass,
                replica_groups=[list(range(N_CORES))],
                ins=[a2a1_in.opt()], outs=[a2a1_out.opt()],
            )
            nc.gpsimd.collective_compute(
                "AllToAll", mybir.AluOpType.bypass,
                replica_groups=[list(range(N_CORES))],
                ins=[a2av_in.opt()], outs=[a2av_out.opt()],
            )

        # ============ attention super-phase ============
        with tc.tile_pool(name="attnsb", bufs=1) as attnsb:
            # x + bo (residual base), done while collective runs
            for i in range(TT):
                nc.vector.tensor_add(out=x_sb[i][:], in0=x_sb[i][:],
                                     in1=bo_bc[:])

            # qkv_w[w]: [128, 1024] = qT | kT for window w
            qkv_w = [attnsb.tile([P, 2 * 512], FP8, tag=f"qkv{w}",
                                 name=f"qkv{w}") for w in range(NW)]
            # v_aug per head per key-tile pair: [128 keys, 2, 96]
            # (64 v cols + ones col + zero pad; DR stationary width must be
            # a multiple of 32)
            vgA = [attnsb.tile([P, 192], FP8, tag=f"vgA{i}", name=f"vgA{i}")
                   for i in range(16)]
            vgB = [attnsb.tile([P, 192], FP8, tag=f"vgB{i}", name=f"vgB{i}")
                   for i in range(16)]
            for i in range(16):
                for vg in (vgA[i], vgB[i]):
                    nc.vector.memset(vg[:], 0.0)
                    nc.vector.memset(vg[:, HD:HD + 1], 1.0)
                    nc.vector.memset(vg[:, 96 + HD:96 + HD + 1], 1.0)

            # prefetch all window loads up front; v goes straight into
            # the v_aug pair tiles (token-major already)
            for w in range(NW):
                nc.sync.dma_start(
                    out=qkv_w[w][:].rearrange("p (blk c) -> p blk c",
                                              blk=2),
                    in_=a2a1_out[w * 2 * P:(w + 1) * 2 * P, :].rearrange(
                        "(blk p) c -> p blk c", p=P))
                for si in range(4):
                    vsrc = a2av_out[w * P + 32 * si:
                                    w * P + 32 * (si + 1),
                                    :].rearrange("r (s f) -> (r s) f", s=4)
                    pi = w * 2 + si // 2
                    col = (si % 2) * 96
                    nc.gpsimd.dma_start(out=vgA[pi][:, col:col + HD],
                                        in_=vsrc[:, 0:HD])
                    nc.gpsimd.dma_start(out=vgB[pi][:, col:col + HD],
                                        in_=vsrc[:, HD:P])

            o_sbs = [attnsb.tile([65, 512], BF16, tag=f"osb{i}",
                                 name=f"osb{i}") for i in range(16)]
            with tc.tile_pool(name="scp", bufs=3, space="PSUM") as scp, \
                 tc.tile_pool(name="ptp", bufs=6) as ptp, \
                 tc.tile_pool(name="lop", bufs=1, space="PSUM") as lop, \
                 tc.tile_pool(name="nrm2", bufs=3) as nrm2:

                def attn_window(b, tcl):
                    tch = b * 4 + tcl
                    n_s = 4 * (tcl + 1)
                    np2 = n_s // 2
                    qT = qkv_w[tch]
                    o_ps = {}
                    for hh in range(2):
                        o_ps[hh] = lop.tile([96, 512], F32, tag=f"ops{hh}",
                                            name=f"o_ps{hh}")
                    pend = []

                    def emit_sc(j, hh):
                        rows = slice(hh * HD, (hh + 1) * HD)
                        sc = scp.tile([P, 1024], F32, tag="sc", name="sc")
                        diag = []
                        for half in range(2):
                            si = 2 * j + half
                            kw = b * 4 + si // 4
                            kc = 512 + (si % 4) * P
                            nc.tensor.matmul(
                                sc[:, half * 512:(half + 1) * 512],
                                qkv_w[kw][rows, kc:kc + P],
                                qT[rows, 0:512],
                                start=True, stop=True,
                                tile_position=(hh * HD, 0))
                            if si // 4 == tcl:
                                diag.append((half, (si % 4) * P))
                        p = ptp.tile([P, 1024], FP8, tag="p", name="p")
                        if not diag:
                            nc.scalar.activation(
                                out=p[:], in_=sc[:],
                                func=AF.Exp, scale=SCALE)
                        else:
                            for half, c0 in diag:
                                base = half * 512
                                nc.vector.tensor_add(
                                    out=sc[:, base + c0:base + c0 + P],
                                    in0=sc[:, base + c0:base + c0 + P],
                                    in1=mask_tri[:])
                                if c0:
                                    nc.vector.memset(
                                        p[:, base:base + c0], 0.0)
                                nc.scalar.activation(
                                    out=p[:, base + c0:base + 512],
                                    in_=sc[:, base + c0:base + 512],
                                    func=AF.Exp, scale=SCALE)
                        pend.append((p, j, hh))

                    def emit_av():
                        p, j, hh = pend.pop(0)
                        vg = vgA if hh == 0 else vgB
                        nc.tensor.matmul(
                            o_ps[hh][:], pair_view(vg[b * 8 + j], 96),
                            pair_view(p, 512),
                            start=(j == 0), stop=(j == np2 - 1),
                            perf_mode=DR)

                    for j in range(np2):
                        emit_sc(j, 0)
                        emit_sc(j, 1)
                        while len(pend) > 2:
                            emit_av()
                    while pend:
                        emit_av()
                    for hh in range(2):
                        osb = o_sbs[tch * 2 + hh]
                        nc.vector.tensor_copy(osb[:], o_ps[hh][0:65, :])

                def normalize_entry(idx):
                    osb = o_sbs[idx]
                    lf = nrm2.tile([65, 512], F32, tag="lf", name="lf")
                    nc.vector.tensor_copy(lf[64:65, :], osb[64:65, :])
                    l0 = nrm2.tile([1, 512], F32, tag="l0", name="l0")
                    nc.sync.dma_start(out=l0[:], in_=lf[64:65, :])
                    lr = nrm2.tile([1, 512], F32, tag="lr", name="lr")
                    nc.vector.reciprocal_approx_fast(out=lr[:], in_=l0[:])
                    lbc = nrm2.tile([HD, 512], F32, tag="lbc", name="lbc")
                    nc.gpsimd.partition_broadcast(lbc[:], lr[0:1, :])
                    o_n = nrm2.tile([HD, 512], FP8, tag="on", name="o_n")
                    nc.vector.tensor_mul(out=o_n[:], in0=osb[0:HD, :],
                                         in1=lbc[:])
                    nc.sync.dma_start(
                        out=a2a2_in[idx * HD:(idx + 1) * HD, :],
                        in_=o_n[:])

                for b in range(B):
                    for tcl in range(4):
                        attn_window(b, tcl)
                for idx in range(16):
                    normalize_entry(idx)

            if stub_cc:
                nc.sync.dma_start(out=a2a2_out[:, :], in_=a2a2_in[:, :])
            else:
                nc.gpsimd.collective_compute(
                    "AllToAll", mybir.AluOpType.bypass,
                    replica_groups=[list(range(N_CORES))],
                    ins=[a2a2_in.opt()], outs=[a2a2_out.opt()],
                )

            # ---- Wo projection (DoubleRow) + residual ----
            with tc.tile_pool(name="wosb", bufs=1) as wosb, \
                 tc.tile_pool(name="wop", bufs=1, space="PSUM") as wop:
                o8 = [wosb.tile([P, 1024], FP8, tag=f"o8_{c}", name=f"o8_{c}")
                      for c in range(CP)]
                for c in range(CP):
                    nc.sync.dma_start(
                        out=o8[c][:].rearrange("p (two c) -> p two c", two=2),
                        in_=a2a2_out[c * 256:(c + 1) * 256, :].rearrange(
                            "(two p) c -> p two c", p=P))
                pso = [wop.tile([P, 512], F32, tag=f"wo{i}", name=f"wo{i}")
                       for i in range(8)]
                for c in range(CP):
                    for tt in range(TT):
                        lhs = pair_view(o8[c], 512)[:, :, tt * P:(tt + 1) * P]
                        for dc in range(2):
                            nc.tensor.matmul(
                                pso[tt * 2 + dc][:], lhs,
                                pair_view(wos[c], 1024)[:, :,
                                                        dc * 512:
                                                        (dc + 1) * 512],
                                start=(c == 0), stop=(c == CP - 1),
                                perf_mode=DR)
                for tt in range(TT):
                    for dc in range(2):
                        sl = slice(dc * 512, (dc + 1) * 512)
                        nc.vector.tensor_add(out=r1[tt][:, sl],
                                             in0=pso[tt * 2 + dc][:],
                                             in1=x_sb[tt][:, sl])

            # ---- LN2 + transpose (bf16, kt-major) ----
            h2T = [attnsb.tile([P, 512], BF16, tag=f"h2T{k}",
                               name=f"h2T{k}") for k in range(KT)]
            with tc.tile_pool(name="h2p", bufs=1) as h2p:
                h2 = layernorm_tiles(r1, h2p, "h2")
                with tc.tile_pool(name="tr2p", bufs=4, space="PSUM") as tr2p:
                    for i in range(TT):
                        for kt in range(KT):
                            pt2 = tr2p.tile([P, P], BF16, tag="tr2",
                                            name="pt2")
                            nc.tensor.transpose(
                                pt2[:], h2[i][:, kt * P:(kt + 1) * P],
                                ident_b[:])
                            nc.vector.tensor_scalar(
                                out=h2T[kt][:, i * P:(i + 1) * P],
                                in0=pt2[:],
                                scalar1=g2_s[:, kt:kt + 1],
                                scalar2=b2l_s[:, kt:kt + 1],
                                op0=mybir.AluOpType.mult,
                                op1=mybir.AluOpType.add,
                            )

            # r1 + b2 (residual base for ffn out)
            for i in range(TT):
                nc.vector.tensor_add(out=r1[i][:], in0=r1[i][:],
                                     in1=b2_bc[:])

            # ---- FFN (bf16, streamed weights) ----
            ff1 = [attnsb.tile([P, 512], BF16, tag=f"ff1_{k}",
                               name=f"ff1_{k}") for k in range(FT)]
            with tc.tile_pool(name="w1st", bufs=4) as w1st, \
                 tc.tile_pool(name="ff1pp", bufs=4, space="PSUM") as ff1pp:
                for j in range(FJ):
                    w1t = w1st.tile([P, 2048], BF16, tag="w1t", name="w1t")
                    nc.gpsimd.dma_start(out=w1t[:],
                                        in_=w1b[j * P:(j + 1) * P, :])
                    for half in range(2):
                        ft = 2 * j + half
                        ps = ff1pp.tile([P, 512], F32, tag="ff1", name="ps")
                        for kt in range(KT):
                            nc.tensor.matmul(
                                ps[:],
                                w1t[:, half * 1024 + kt * P:
                                    half * 1024 + (kt + 1) * P],
                                h2T[kt][:],
                                start=(kt == 0), stop=(kt == KT - 1))
                        nc.scalar.activation(
                            out=ff1[ft][:],
                            in_=ps[:], func=AF.Relu, bias=b1_s[:, ft:ft + 1])

            with tc.tile_pool(name="w2st", bufs=4) as w2st, \
                 tc.tile_pool(name="outp", bufs=4) as outp, \
                 tc.tile_pool(name="ff2p", bufs=1, space="PSUM") as ff2p:
                ps2 = [ff2p.tile([P, 512], F32, tag=f"ff2_{i}",
                                 name=f"ff2_{i}") for i in range(8)]
                w2ts = []
                for j in range(2):
                    w2t = w2st.tile([P, 2048], BF16, tag="w2t", name="w2t")
                    nc.scalar.dma_start(out=w2t[:],
                                        in_=w2b[j * P:(j + 1) * P, :])
                    w2ts.append(w2t)
                for j in range(FJ):
                    if j < 2:
                        w2t = w2ts[j]
                    else:
                        w2t = w2st.tile([P, 2048], BF16, tag="w2t",
                                        name="w2t")
                        nc.gpsimd.dma_start(out=w2t[:],
                                            in_=w2b[j * P:(j + 1) * P, :])
                    for half in range(2):
                        kt = 2 * j + half
                        for tt in range(TT):
                            lhs = ff1[kt][:, tt * P:(tt + 1) * P]
                            for dc in range(2):
                                nc.tensor.matmul(
                                    ps2[tt * 2 + dc][:], lhs,
                                    w2t[:, half * 1024 + dc * 512:
                                        half * 1024 + (dc + 1) * 512],
                                    start=(kt == 0), stop=(kt == FT - 1))
                for tt in range(TT):
                    for dc in range(2):
                        sl = slice(dc * 512, (dc + 1) * 512)
                        nc.vector.tensor_add(out=x_out[tt][:, sl],
                                             in0=ps2[tt * 2 + dc][:],
                                             in1=r1[tt][:, sl])
                    if y is not None:
                        nc.sync.dma_start(out=y[tt * P:(tt + 1) * P, :],
                                          in_=x_out[tt][:])


def _dr_pack(w, dt=None):
    """[K, N] -> DoubleRow pair layout [K/2, 2N]: row c*128+p,
    col i*N+n  holds  w[c*256 + i*128 + p, n]."""
    import ml_dtypes
    if dt is None:
        dt = ml_dtypes.float8_e4m3fn
    K, N = w.shape
    nch = K // 256
    t = w.reshape(nch, 2, 128, N).transpose(0, 2, 1, 3).reshape(K // 2, 2 * N)
    return np.ascontiguousarray(t.astype(dt))


def _w1_pack(w1):
    """[D, F] -> streamed lhsT pair tiles: row j*128+p,
    col half*1024 + kt*128 + c  holds  W1[kt*128+p, (2j+half)*128+c]."""
    import ml_dtypes
    t = w1.reshape(KT, P, FJ, 2, P).transpose(2, 1, 3, 0, 4).reshape(
        F // 2, 2 * D)
    return np.ascontiguousarray(t.astype(ml_dtypes.bfloat16))


def _shard_inputs(inputs):
    import ml_dtypes
    x = np.ascontiguousarray(
        np.asarray(inputs["x"], np.float32).reshape(NTOK, D))
    Wq = np.asarray(inputs["Wq"], np.float32).transpose(1, 0, 2).reshape(D, D)
    Wk = np.asarray(inputs["Wk"], np.float32).transpose(1, 0, 2).reshape(D, D)
    Wv = np.asarray(inputs["Wv"], np.float32).transpose(1, 0, 2).reshape(D, D)
    wqkv8 = np.concatenate([_dr_pack(Wq), _dr_pack(Wk), _dr_pack(Wv)], axis=0)
    com = dict(
        wqkv8=wqkv8,
        wo8=_dr_pack(np.asarray(inputs["Wo"], np.float32)),
        w1b=_w1_pack(np.asarray(inputs["W1"], np.float32)),
        w2b=_dr_pack(np.asarray(inputs["W2"], np.float32),
                     dt=ml_dtypes.bfloat16),
        bo=np.asarray(inputs["bo"], np.float32),
        b1=np.asarray(inputs["b1"], np.float32),
        b2=np.asarray(inputs["b2"], np.float32),
        ln1_g=np.asarray(inputs["ln1_g"], np.float32),
        ln1_b=np.asarray(inputs["ln1_b"], np.float32),
        ln2_g=np.asarray(inputs["ln2_g"], np.float32),
        ln2_b=np.asarray(inputs["ln2_b"], np.float32),
    )
    maps = []
    for c in range(N_CORES):
        m = dict(com)
        m["x"] = x[c * S:(c + 1) * S]
        maps.append(m)
    return maps


def _get_nc():
    if "nc" not in _CACHE:
        _CACHE["nc"] = _build()
    return _CACHE["nc"]


def _run(in_maps):
    from concourse.bass_utils import run_bass_kernel_spmd
    nc = _get_nc()
    res = run_bass_kernel_spmd(nc, in_maps, core_ids=list(range(N_CORES)))
    return res.results


def kernel(**inputs):
    in_maps = _shard_inputs(inputs)
    results = _run(in_maps)
    out = np.concatenate([results[c]["y"] for c in range(N_CORES)], axis=0)
    return out.reshape(B, T, D)


# revision 11
# speedup vs baseline: 1.0189x; 1.0066x over previous
"""Distributed Trainium2 kernel for a dense transformer block (v2).

Sharding (8 cores, one chip):
  - Each core owns S=512 of the 4096 tokens (sequence-parallel LN/FFN/residual)
    and one head-pair (2 of 16 heads) for attention.
  - QKV projections computed locally on own tokens for ALL heads (fp8
    DoubleRow), then ONE fused AllToAll redistributes q/k/v to head-owners.
  - Attention per head-pair over all tokens; softmax denominator folded into
    the AV matmul via a ones-column on V; AV runs fp8 DoubleRow over key-tile
    pairs; the scores->exp->AV chain is software-pipelined so the PE never
    waits on the activation engine.  Normalized outputs return to
    token-owners via a second (small) fp8 AllToAll.
  - Wo runs fp8 DoubleRow; the FFN runs bf16 (fp8 there fails the accuracy
    budget) with weights streamed from DRAM in 512KB chunks on the gpsimd
    DMA queue.
"""

import sys

sys.path.insert(0, "/opt/trn_rl_repo")

import numpy as np

import concourse.bacc as bacc
import concourse.bass as bass
import concourse.tile as tile
from concourse import mybir
from concourse.masks import make_identity

F32 = mybir.dt.float32
BF16 = mybir.dt.bfloat16
FP8 = mybir.dt.float8e4
AF = mybir.ActivationFunctionType
DR = mybir.MatmulPerfMode.DoubleRow

N_CORES = 8
B, T, D, H = 2, 2048, 1024, 16
HD = D // H            # 64
NTOK = B * T           # 4096
S = NTOK // N_CORES    # 512 tokens per core
F = 4 * D              # 4096
EPS = 1e-5
SCALE = float(D) ** -0.5
MASK_VAL = -30000.0
P = 128

KT = D // P            # 8 feature tiles
TT = S // P            # 4 token tiles in the shard
NW = N_CORES           # 8 global 512-token windows
FT = F // P            # 32 ffn-hidden tiles
CP = D // 256          # 4 contraction chunk-pairs over D
FJ = F // 256          # 16 chunk-pairs over F
BLK = 3 * P            # 384 rows per a2a1 block (q,k,v)

_CACHE = {}


def _build(n_chain=1, stub_cc=False):
    nc = bacc.Bacc("TRN2", target_bir_lowering=False, debug=False,
                   num_devices=N_CORES)

    x = nc.dram_tensor("x", [S, D], F32, kind="ExternalInput")
    wqkv8 = nc.dram_tensor("wqkv8", [3 * 512, 2048], FP8, kind="ExternalInput")
    wo8 = nc.dram_tensor("wo8", [512, 2048], FP8, kind="ExternalInput")
    w1b = nc.dram_tensor("w1b", [F // 2, 2 * D], BF16, kind="ExternalInput")
    w2b = nc.dram_tensor("w2b", [F // 2, 2 * D], BF16, kind="ExternalInput")
    bo = nc.dram_tensor("bo", [D], F32, kind="ExternalInput")
    b1 = nc.dram_tensor("b1", [F], F32, kind="ExternalInput")
    b2 = nc.dram_tensor("b2", [D], F32, kind="ExternalInput")
    ln1_g = nc.dram_tensor("ln1_g", [D], F32, kind="ExternalInput")
    ln1_b = nc.dram_tensor("ln1_b", [D], F32, kind="ExternalInput")
    ln2_g = nc.dram_tensor("ln2_g", [D], F32, kind="ExternalInput")
    ln2_b = nc.dram_tensor("ln2_b", [D], F32, kind="ExternalInput")
    y = nc.dram_tensor("y", [S, D], F32, kind="ExternalOutput")
    global _W
    _W = dict(wqkv8=wqkv8, wo8=wo8, w1b=w1b, w2b=w2b, bo=bo, b1=b1, b2=b2,
              ln1_g=ln1_g, ln1_b=ln1_b, ln2_g=ln2_g, ln2_b=ln2_b)

    with tile.TileContext(nc) as tc:
      with tc.tile_pool(name="xch", bufs=1) as xchp, \
           tc.tile_pool(name="gw", bufs=1) as gwp:
        xsets = [[xchp.tile([P, D], F32, tag=f"x{s}_{i}", name=f"x{s}_{i}")
                  for i in range(TT)] for s in range(2)]
        wsb = [gwp.tile([P, 8192], FP8, tag=f"w{pj}", name=f"w{pj}")
               for pj in range(3)]
        for pj in range(3):
            nc.gpsimd.dma_start(
                out=wsb[pj][:].rearrange("p (c n) -> p c n", c=CP),
                in_=wqkv8[pj * 512:(pj + 1) * 512, :].rearrange(
                    "(c p) n -> p c n", p=P))
        wos = [gwp.tile([P, 2048], FP8, tag=f"wos{c}", name=f"wos{c}")
               for c in range(CP)]
        for c in range(CP):
            nc.gpsimd.dma_start(out=wos[c][:], in_=wo8[c * P:(c + 1) * P, :])
        for _ci in range(n_chain):
            _emit_body(nc, tc,
                       x if _ci == 0 else None,
                       y if _ci == n_chain - 1 else None,
                       xsets[_ci % 2], xsets[(_ci + 1) % 2],
                       wsb, wos, _ci, stub_cc)

    nc.compile()
    return nc


def _emit_body(nc, tc, x, y, x_sb, x_out, wsb, wos, ci, stub_cc=False):
    wqkv8, wo8, w1b, w2b = _W["wqkv8"], _W["wo8"], _W["w1b"], _W["w2b"]
    bo, b1, b2 = _W["bo"], _W["b1"], _W["b2"]
    ln1_g, ln1_b = _W["ln1_g"], _W["ln1_b"]
    ln2_g, ln2_b = _W["ln2_g"], _W["ln2_b"]

    with tc.tile_pool(name="dram", bufs=1, space="DRAM") as dram, \
         tc.tile_pool(name="const", bufs=1) as const, \
         tc.tile_pool(name="persist", bufs=1) as persist:

        a2a1_in = dram.tile([NW * 2 * P, S], FP8, tag="a1i", name="a2a1_in")
        a2a1_out = dram.tile([NW * 2 * P, S], FP8, tag="a1o",
                             name="a2a1_out")
        a2av_in = dram.tile([NW * P, S], FP8, tag="avi", name="a2av_in")
        a2av_out = dram.tile([NW * P, S], FP8, tag="avo", name="a2av_out")
        a2a2_in = dram.tile([NW * P, S], FP8, tag="a2i", name="a2a2_in")
        a2a2_out = dram.tile([NW * P, S], FP8, tag="a2o", name="a2a2_out")

        # ---- constants ----
        ident_b = const.tile([P, P], BF16, tag="identb", name="ident_b")
        make_identity(nc, ident_b)

        eps_t = const.tile([P, 1], F32, tag="eps", name="eps_t")
        nc.vector.memset(eps_t[:], EPS)

        # triangular mask for the diagonal 128x128 sub-block:
        # m[r, q] = 0 if r <= q else MASK_VAL
        mask_tri = const.tile([P, P], F32, tag="mtri", name="mask_tri")
        nc.gpsimd.memset(mask_tri[:], 0.0)
        nc.gpsimd.affine_select(
            out=mask_tri[:], in_=mask_tri[:],
            compare_op=mybir.AluOpType.is_ge,
            fill=MASK_VAL, base=0,
            pattern=[[1, P]], channel_multiplier=-1,
        )

        g1_s = const.tile([P, KT], F32, tag="g1", name="g1_s")
        b1l_s = const.tile([P, KT], F32, tag="b1l", name="b1l_s")
        g2_s = const.tile([P, KT], F32, tag="g2", name="g2_s")
        b2l_s = const.tile([P, KT], F32, tag="b2l", name="b2l_s")
        nc.scalar.dma_start(out=g1_s[:],
                            in_=ln1_g.ap().rearrange("(k p) -> p k", p=P))
        nc.scalar.dma_start(out=b1l_s[:],
                            in_=ln1_b.ap().rearrange("(k p) -> p k", p=P))
        nc.scalar.dma_start(out=g2_s[:],
                            in_=ln2_g.ap().rearrange("(k p) -> p k", p=P))
        nc.scalar.dma_start(out=b2l_s[:],
                            in_=ln2_b.ap().rearrange("(k p) -> p k", p=P))

        b1_s = const.tile([P, FT], F32, tag="b1s", name="b1_s")
        nc.scalar.dma_start(out=b1_s[:],
                            in_=b1.ap().rearrange("(k p) -> p k", p=P))

        bo_bc = const.tile([P, D], F32, tag="bo_bc", name="bo_bc")
        b2_bc = const.tile([P, D], F32, tag="b2_bc", name="b2_bc")
        nc.scalar.dma_start(out=bo_bc[:], in_=bo.ap().partition_broadcast(P))
        nc.scalar.dma_start(out=b2_bc[:], in_=b2.ap().partition_broadcast(P))

        # persistent: r1; x shard tiles passed in (SBUF-resident chain)
        if x is not None:
            for i in range(TT):
                nc.sync.dma_start(out=x_sb[i][:],
                                  in_=x[i * P:(i + 1) * P, :])
        r1 = [persist.tile([P, D], F32, tag=f"r1_{i}", name=f"r1_{i}")
              for i in range(TT)]

        def layernorm_tiles(src_tiles, pool, out_name):
            """LN over the free axis -> normalized bf16 [t,d] tiles
            (gamma/beta folded in later at transpose-evict)."""
            out = []
            with tc.tile_pool(name=f"ln_{out_name}", bufs=2) as lnp:
                for i, xt in enumerate(src_tiles):
                    st = lnp.tile([P, 2, 6], F32, tag="stats", name="st")
                    xr = xt[:].rearrange("p (s f) -> p s f", s=2)
                    for sg in range(2):
                        nc.vector.bn_stats(out=st[:, sg, :], in_=xr[:, sg, :])
                    mv = lnp.tile([P, 2], F32, tag="mv", name="mv")
                    nc.vector.bn_aggr(out=mv[:], in_=st[:])
                    rstd = lnp.tile([P, 1], F32, tag="rstd", name="rstd")
                    nc.scalar.activation(out=rstd[:], in_=mv[:, 1:2],
                                         func=AF.Sqrt, bias=eps_t[:],
                                         scale=1.0)
                    nc.vector.reciprocal(out=rstd[:], in_=rstd[:])
                    o = pool.tile([P, D], BF16, tag=f"{out_name}{i}",
                                  name=f"{out_name}{i}")
                    nc.vector.tensor_scalar(
                        out=o[:], in0=xt[:],
                        scalar1=mv[:, 0:1], scalar2=rstd[:],
                        op0=mybir.AluOpType.subtract,
                        op1=mybir.AluOpType.mult,
                    )
                    out.append(o)
            return out

        def pair_view(t, width):
            """[128, 2*width] tile -> [128, 2, width] DoubleRow view."""
            return t[:].rearrange("p (two n) -> p two n", two=2)

        # ================= QKV phase =================
        with tc.tile_pool(name="qkvsb", bufs=1) as qkvsb:
            # LN1 + transpose, folding gamma/beta, writing fp8 pair tiles
            h8 = [qkvsb.tile([P, 1024], FP8, tag=f"h8_{c}", name=f"h8_{c}")
                  for c in range(CP)]
            with tc.tile_pool(name="xlnp", bufs=1) as xlnp:
                xln = layernorm_tiles(x_sb, xlnp, "xln")
                with tc.tile_pool(name="tr1p", bufs=4, space="PSUM") as trpp:
                    for i in range(TT):
                        for kt in range(KT):
                            pt = trpp.tile([P, P], BF16, tag="tr", name="pt")
                            nc.tensor.transpose(
                                pt[:], xln[i][:, kt * P:(kt + 1) * P],
                                ident_b[:])
                            nc.vector.tensor_scalar(
                                out=h8[kt // 2][:, (kt % 2) * 512 + i * P:
                                                (kt % 2) * 512 + (i + 1) * P],
                                in0=pt[:],
                                scalar1=g1_s[:, kt:kt + 1],
                                scalar2=b1l_s[:, kt:kt + 1],
                                op0=mybir.AluOpType.mult,
                                op1=mybir.AluOpType.add,
                            )

            def wqkv_view(pj, c, ob):
                v = wsb[pj][:].rearrange("p (c two n) -> p c two n",
                                         c=CP, two=2)
                return v[:, c, :, ob * P:(ob + 1) * P]

            with tc.tile_pool(name="qkvst", bufs=3) as qst, \
                 tc.tile_pool(name="qkvp", bufs=2, space="PSUM") as qkvp, \
                 tc.tile_pool(name="vtp", bufs=2, space="PSUM") as vtp:
                pend_v = []

                def emit_vtr():
                    stv, ob = pend_v.pop(0)
                    for j4 in range(4):
                        pv = vtp.tile([P, P], BF16, tag="pv", name="pv")
                        nc.tensor.transpose(pv[:], stv[:, j4 * P:(j4 + 1) * P],
                                            ident_b[:])
                        v8 = qst.tile([P, P], FP8, tag="v8", name="v8")
                        nc.vector.tensor_copy(v8[:], pv[:])
                        nc.sync.dma_start(
                            out=a2av_in[ob * P + 32 * j4:
                                        ob * P + 32 * (j4 + 1),
                                        :].rearrange("r (s f) -> (r s) f",
                                                     s=4),
                            in_=v8[:])

                for ob in range(NW):
                    pss = [qkvp.tile([P, 512], F32, tag=f"pp{pj}",
                                     name=f"pp{pj}") for pj in range(3)]
                    for c in range(CP):
                        for pj in range(3):
                            nc.tensor.matmul(
                                pss[pj][:],
                                wqkv_view(pj, c, ob),
                                pair_view(h8[c], 512),
                                start=(c == 0), stop=(c == CP - 1),
                                perf_mode=DR)
                    st = qst.tile([P, 1024], FP8, tag="st", name="st")
                    nc.scalar.copy(st[:, 0:512], pss[0][:])
                    nc.scalar.copy(st[:, 512:1024], pss[1][:])
                    nc.sync.dma_start(
                        out=a2a1_in[ob * 2 * P:(ob + 1) * 2 * P,
                                    :].rearrange(
                            "(blk p) c -> p blk c", p=P),
                        in_=st[:].rearrange("p (blk c) -> p blk c", blk=2))
                    stv = qst.tile([P, 512], BF16, tag="stv", name="stv")
                    nc.vector.tensor_copy(stv[:], pss[2][:])
                    pend_v.append((stv, ob))
                    if len(pend_v) > 1:
                        emit_vtr()
                while pend_v:
                    emit_vtr()

        if stub_cc:
            nc.sync.dma_start(out=a2a1_out[:, :], in_=a2a1_in[:, :])
            nc.sync.dma_start(out=a2av_out[:, :], in_=a2av_in[:, :])
        else:
            nc.gpsimd.collective_compute(
                "AllToAll", mybir.AluOpType.bypass,
                replica_groups=[list(range(N_CORES))],
                ins=[a2a1_in.opt()], outs=[a2a1_out.opt()],
            )
            nc.gpsimd.collective_compute(
                "AllToAll", mybir.AluOpType.bypass,
                replica_groups=[list(range(N_CORES))],
                ins=[a2av_in.opt()], outs=[a2av_out.opt()],
            )

        # ============ attention super-phase ============
        with tc.tile_pool(name="attnsb", bufs=1) as attnsb:
            # x + bo (residual base), done while collective runs
            for i in range(TT):
                nc.vector.tensor_add(out=x_sb[i][:], in0=x_sb[i][:],
                                     in1=bo_bc[:])

            # qkv_w[w]: [128, 1024] = qT | kT for window w
            qkv_w = [attnsb.tile([P, 2 * 512], FP8, tag=f"qkv{w}",
                                 name=f"qkv{w}") for w in range(NW)]
            # v_aug per head per key-tile pair: [128 keys, 2, 96]
            # (64 v cols + ones col + zero pad; DR stationary width must be
            # a multiple of 32)
            vgA = [attnsb.tile([P, 192], FP8, tag=f"vgA{i}", name=f"vgA{i}")
                   for i in range(16)]
            vgB = [attnsb.tile([P, 192], FP8, tag=f"vgB{i}", name=f"vgB{i}")
                   for i in range(16)]
            for i in range(16):
                for vg in (vgA[i], vgB[i]):
                    nc.vector.memset(vg[:], 0.0)
                    nc.vector.memset(vg[:, HD:HD + 1], 1.0)
                    nc.vector.memset(vg[:, 96 + HD:96 + HD + 1], 1.0)

            # prefetch all window loads up front; v goes straight into
            # the v_aug pair tiles (token-major already)
            for w in range(NW):
                nc.sync.dma_start(
                    out=qkv_w[w][:].rearrange("p (blk c) -> p blk c",
                                              blk=2),
                    in_=a2a1_out[w * 2 * P:(w + 1) * 2 * P, :].rearrange(
                        "(blk p) c -> p blk c", p=P))
                for si in range(4):
                    vsrc = a2av_out[w * P + 32 * si:
                                    w * P + 32 * (si + 1),
                                    :].rearrange("r (s f) -> (r s) f", s=4)
                    pi = w * 2 + si // 2
                    col = (si % 2) * 96
                    nc.gpsimd.dma_start(out=vgA[pi][:, col:col + HD],
                                        in_=vsrc[:, 0:HD])
                    nc.gpsimd.dma_start(out=vgB[pi][:, col:col + HD],
                                        in_=vsrc[:, HD:P])

            o_sbs = [attnsb.tile([65, 512], BF16, tag=f"osb{i}",
                                 name=f"osb{i}") for i in range(16)]
            with tc.tile_pool(name="scp", bufs=3, space="PSUM") as scp, \
                 tc.tile_pool(name="ptp", bufs=6) as ptp, \
                 tc.tile_pool(name="lop", bufs=1, space="PSUM") as lop, \
                 tc.tile_pool(name="nrm2", bufs=3) as nrm2:

                pend = []
                o_ps_map = {}

                def emit_av():
                    p, j, hh, np2_, b_, tch_ = pend.pop(0)
                    key = (tch_, hh)
                    if j == 0:
                        o_ps_map[key] = lop.tile([96, 512], F32,
                                                 tag=f"ops{hh}",
                                                 name=f"o_ps{hh}")
                    o_ps_h = o_ps_map[key]
                    vg = vgA if hh == 0 else vgB
                    nc.tensor.matmul(
                        o_ps_h[:], pair_view(vg[b_ * 8 + j], 96),
                        pair_view(p, 512),
                        start=(j == 0), stop=(j == np2_ - 1),
                        perf_mode=DR)
                    if j == np2_ - 1:
                        osb = o_sbs[tch_ * 2 + hh]
                        nc.vector.tensor_copy(osb[:], o_ps_h[0:65, :])

                def attn_window(b, tcl):
                    tch = b * 4 + tcl
                    n_s = 4 * (tcl + 1)
                    np2 = n_s // 2
                    qT = qkv_w[tch]

                    def emit_sc(j, hh):
                        rows = slice(hh * HD, (hh + 1) * HD)
                        sc = scp.tile([P, 1024], F32, tag="sc", name="sc")
                        diag = []
                        for half in range(2):
                            si = 2 * j + half
                            kw = b * 4 + si // 4
                            kc = 512 + (si % 4) * P
                            nc.tensor.matmul(
                                sc[:, half * 512:(half + 1) * 512],
                                qkv_w[kw][rows, kc:kc + P],
                                qT[rows, 0:512],
                                start=True, stop=True,
                                tile_position=(hh * HD, 0))
                            if si // 4 == tcl:
                                diag.append((half, (si % 4) * P))
                        p = ptp.tile([P, 1024], FP8, tag="p", name="p")
                        if not diag:
                            nc.scalar.activation(
                                out=p[:], in_=sc[:],
                                func=AF.Exp, scale=SCALE)
                        else:
                            for half, c0 in diag:
                                base = half * 512
                                nc.vector.tensor_add(
                                    out=sc[:, base + c0:base + c0 + P],
                                    in0=sc[:, base + c0:base + c0 + P],
                                    in1=mask_tri[:])
                                if c0:
                                    nc.vector.memset(
                                        p[:, base:base + c0], 0.0)
                                nc.scalar.activation(
                                    out=p[:, base + c0:base + 512],
                                    in_=sc[:, base + c0:base + 512],
                                    func=AF.Exp, scale=SCALE)
                        pend.append((p, j, hh, np2, b, tch))

                    for j in range(np2):
                        emit_sc(j, 0)
                        emit_sc(j, 1)
                        while len(pend) > 2:
                            emit_av()

                def normalize_entry(idx):
                    osb = o_sbs[idx]
                    lf = nrm2.tile([65, 512], F32, tag="lf", name="lf")
                    nc.vector.tensor_copy(lf[64:65, :], osb[64:65, :])
                    l0 = nrm2.tile([1, 512], F32, tag="l0", name="l0")
                    nc.sync.dma_start(out=l0[:], in_=lf[64:65, :])
                    lr = nrm2.tile([1, 512], F32, tag="lr", name="lr")
                    nc.vector.reciprocal_approx_fast(out=lr[:], in_=l0[:])
                    lbc = nrm2.tile([HD, 512], F32, tag="lbc", name="lbc")
                    nc.gpsimd.partition_broadcast(lbc[:], lr[0:1, :])
                    o_n = nrm2.tile([HD, 512], FP8, tag="on", name="o_n")
                    nc.vector.tensor_mul(out=o_n[:], in0=osb[0:HD, :],
                                         in1=lbc[:])
                    nc.sync.dma_start(
                        out=a2a2_in[idx * HD:(idx + 1) * HD, :],
                        in_=o_n[:])

                for b in range(B):
                    for tcl in range(4):
                        attn_window(b, tcl)
                while pend:
                    emit_av()
                for idx in range(16):
                    normalize_entry(idx)

            if stub_cc:
                nc.sync.dma_start(out=a2a2_out[:, :], in_=a2a2_in[:, :])
            else:
                nc.gpsimd.collective_compute(
                    "AllToAll", mybir.AluOpType.bypass,
                    replica_groups=[list(range(N_CORES))],
                    ins=[a2a2_in.opt()], outs=[a2a2_out.opt()],
                )

            # ---- Wo projection (DoubleRow) + residual ----
            with tc.tile_pool(name="wosb", bufs=1) as wosb, \
                 tc.tile_pool(name="wop", bufs=1, space="PSUM") as wop:
                o8 = [wosb.tile([P, 1024], FP8, tag=f"o8_{c}", name=f"o8_{c}")
                      for c in range(CP)]
                for c in range(CP):
                    nc.sync.dma_start(
                        out=o8[c][:].rearrange("p (two c) -> p two c", two=2),
                        in_=a2a2_out[c * 256:(c + 1) * 256, :].rearrange(
                            "(two p) c -> p two c", p=P))
                pso = [wop.tile([P, 512], F32, tag=f"wo{i}", name=f"wo{i}")
                       for i in range(8)]
                for c in range(CP):
                    for tt in range(TT):
                        lhs = pair_view(o8[c], 512)[:, :, tt * P:(tt + 1) * P]
                        for dc in range(2):
                            nc.tensor.matmul(
                                pso[tt * 2 + dc][:], lhs,
                                pair_view(wos[c], 1024)[:, :,
                                                        dc * 512:
                                                        (dc + 1) * 512],
                                start=(c == 0), stop=(c == CP - 1),
                                perf_mode=DR)
                for tt in range(TT):
                    for dc in range(2):
                        sl = slice(dc * 512, (dc + 1) * 512)
                        nc.vector.tensor_add(out=r1[tt][:, sl],
                                             in0=pso[tt * 2 + dc][:],
                                             in1=x_sb[tt][:, sl])

            # ---- LN2 + transpose (bf16, kt-major) ----
            h2T = [attnsb.tile([P, 512], BF16, tag=f"h2T{k}",
                               name=f"h2T{k}") for k in range(KT)]
            with tc.tile_pool(name="h2p", bufs=1) as h2p:
                h2 = layernorm_tiles(r1, h2p, "h2")
                with tc.tile_pool(name="tr2p", bufs=4, space="PSUM") as tr2p:
                    for i in range(TT):
                        for kt in range(KT):
                            pt2 = tr2p.tile([P, P], BF16, tag="tr2",
                                            name="pt2")
                            nc.tensor.transpose(
                                pt2[:], h2[i][:, kt * P:(kt + 1) * P],
                                ident_b[:])
                            nc.vector.tensor_scalar(
                                out=h2T[kt][:, i * P:(i + 1) * P],
                                in0=pt2[:],
                                scalar1=g2_s[:, kt:kt + 1],
                                scalar2=b2l_s[:, kt:kt + 1],
                                op0=mybir.AluOpType.mult,
                                op1=mybir.AluOpType.add,
                            )

            # r1 + b2 (residual base for ffn out)
            for i in range(TT):
                nc.vector.tensor_add(out=r1[i][:], in0=r1[i][:],
                                     in1=b2_bc[:])

            # ---- FFN (bf16, streamed weights) ----
            ff1 = [attnsb.tile([P, 512], BF16, tag=f"ff1_{k}",
                               name=f"ff1_{k}") for k in range(FT)]
            with tc.tile_pool(name="w1st", bufs=4) as w1st, \
                 tc.tile_pool(name="ff1pp", bufs=4, space="PSUM") as ff1pp:
                for j in range(FJ):
                    w1t = w1st.tile([P, 2048], BF16, tag="w1t", name="w1t")
                    nc.gpsimd.dma_start(out=w1t[:],
                                        in_=w1b[j * P:(j + 1) * P, :])
                    for half in range(2):
                        ft = 2 * j + half
                        ps = ff1pp.tile([P, 512], F32, tag="ff1", name="ps")
                        for kt in range(KT):
                            nc.tensor.matmul(
                                ps[:],
                                w1t[:, half * 1024 + kt * P:
                                    half * 1024 + (kt + 1) * P],
                                h2T[kt][:],
                                start=(kt == 0), stop=(kt == KT - 1))
                        nc.scalar.activation(
                            out=ff1[ft][:],
                            in_=ps[:], func=AF.Relu, bias=b1_s[:, ft:ft + 1])

            with tc.tile_pool(name="w2st", bufs=4) as w2st, \
                 tc.tile_pool(name="outp", bufs=4) as outp, \
                 tc.tile_pool(name="ff2p", bufs=1, space="PSUM") as ff2p:
                ps2 = [ff2p.tile([P, 512], F32, tag=f"ff2_{i}",
                                 name=f"ff2_{i}") for i in range(8)]
                w2ts = []
                for j in range(2):
                    w2t = w2st.tile([P, 2048], BF16, tag="w2t", name="w2t")
                    nc.scalar.dma_start(out=w2t[:],
                                        in_=w2b[j * P:(j + 1) * P, :])
                    w2ts.append(w2t)
                for j in range(FJ):
                    if j < 2:
                        w2t = w2ts[j]
                    else:
                        w2t = w2st.tile([P, 2048], BF16, tag="w2t",
                                        name="w2t")
                        nc.gpsimd.dma_start(out=w2t[:],
                                            in_=w2b[j * P:(j + 1) * P, :])
                    for half in range(2):
                        kt = 2 * j + half
                        for tt in range(TT):
                            lhs = ff1[kt][:, tt * P:(tt + 1) * P]
                            for dc in range(2):
                                nc.tensor.matmul(
                                    ps2[tt * 2 + dc][:], lhs,
                                    w2t[:, half * 1024 + dc * 512:
                                        half * 1024 + (dc + 1) * 512],
                                    start=(kt == 0), stop=(kt == FT - 1))
                for tt in range(TT):
                    for dc in range(2):
                        sl = slice(dc * 512, (dc + 1) * 512)
                        nc.vector.tensor_add(out=x_out[tt][:, sl],
                                             in0=ps2[tt * 2 + dc][:],
                                             in1=r1[tt][:, sl])
                    if y is not None:
                        nc.sync.dma_start(out=y[tt * P:(tt + 1) * P, :],
                                          in_=x_out[tt][:])


def _dr_pack(w, dt=None):
    """[K, N] -> DoubleRow pair layout [K/2, 2N]: row c*128+p,
    col i*N+n  holds  w[c*256 + i*128 + p, n]."""
    import ml_dtypes
    if dt is None:
        dt = ml_dtypes.float8_e4m3fn
    K, N = w.shape
    nch = K // 256
    t = w.reshape(nch, 2, 128, N).transpose(0, 2, 1, 3).reshape(K // 2, 2 * N)
    return np.ascontiguousarray(t.astype(dt))


def _w1_pack(w1):
    """[D, F] -> streamed lhsT pair tiles: row j*128+p,
    col half*1024 + kt*128 + c  holds  W1[kt*128+p, (2j+half)*128+c]."""
    import ml_dtypes
    t = w1.reshape(KT, P, FJ, 2, P).transpose(2, 1, 3, 0, 4).reshape(
        F // 2, 2 * D)
    return np.ascontiguousarray(t.astype(ml_dtypes.bfloat16))


def _shard_inputs(inputs):
    import ml_dtypes
    x = np.ascontiguousarray(
        np.asarray(inputs["x"], np.float32).reshape(NTOK, D))
    Wq = np.asarray(inputs["Wq"], np.float32).transpose(1, 0, 2).reshape(D, D)
    Wk = np.asarray(inputs["Wk"], np.float32).transpose(1, 0, 2).reshape(D, D)
    Wv = np.asarray(inputs["Wv"], np.float32).transpose(1, 0, 2).reshape(D, D)
    wqkv8 = np.concatenate([_dr_pack(Wq), _dr_pack(Wk), _dr_pack(Wv)], axis=0)
    com = dict(
        wqkv8=wqkv8,
        wo8=_dr_pack(np.asarray(inputs["Wo"], np.float32)),
        w1b=_w1_pack(np.asarray(inputs["W1"], np.float32)),
        w2b=_dr_pack(np.asarray(inputs["W2"], np.float32),
                     dt=ml_dtypes.bfloat16),
        bo=np.asarray(inputs["bo"], np.float32),
        b1=np.asarray(inputs["b1"], np.float32),
        b2=np.asarray(inputs["b2"], np.float32),
        ln1_g=np.asarray(inputs["ln1_g"], np.float32),
        ln1_b=np.asarray(inputs["ln1_b"], np.float32),
        ln2_g=np.asarray(inputs["ln2_g"], np.float32),
        ln2_b=np.asarray(inputs["ln2_b"], np.float32),
    )
    maps = []
    for c in range(N_CORES):
        m = dict(com)
        m["x"] = x[c * S:(c + 1) * S]
        maps.append(m)
    return maps


def _get_nc():
    if "nc" not in _CACHE:
        _CACHE["nc"] = _build()
    return _CACHE["nc"]


def _run(in_maps):
    from concourse.bass_utils import run_bass_kernel_spmd
    nc = _get_nc()
    res = run_bass_kernel_spmd(nc, in_maps, core_ids=list(range(N_CORES)))
    return res.results


def kernel(**inputs):
    in_maps = _shard_inputs(inputs)
    results = _run(in_maps)
    out = np.concatenate([results[c]["y"] for c in range(N_CORES)], axis=0)
    return out.reshape(B, T, D)
